# revision 5
# baseline (speedup 1.0000x reference)
"""Multi-head attention (B=1, S=4096, D=1024, H=16) on 8 TRN2 NeuronCores.

Strategy (head-sharded attention + AllToAll context exchange):
  - Host: compact K/V to the unmasked key positions (mask==0 keys contribute
    exactly 0 to softmax numerator and denominator, since the reference's
    -1e9 masking underflows exp to 0.0), transpose activations/weights to
    feature-major, cast matmul operands to bf16.
  - Phase A: core m computes K^T and V projections for its 2 heads over all
    compacted positions; results stay in SBUF (no gather needed).
  - Phase B: Q projection for the same 2 heads over ALL 4096 queries.
  - Phase C: attention for the 2 heads x 4096 queries: scores^T =
    K^T-chunk.T @ Q^T in PSUM ([k,q] layout, per-head via matmul
    tile_position row groups), exp on ScalarE straight out of PSUM (padding
    bias folded into the per-partition activation bias), P@V with a
    ones-augmented V (row 64 = softmax denominators), reciprocal + K=1
    broadcast matmul + multiply to normalize. Per-head context goes to DRAM
    sliced by query block.
  - AllToAll (one per head, 2 MiB, the first overlaps the second head's
    compute) converts head-sharding to query-sharding: afterwards core m
    holds all 16 heads' context for its own 512 queries.
  - Phase D: output projection of the core's 512 rows. The host just
    concatenates the 8 row-slices.
"""

import numpy as np
import ml_dtypes

import concourse.bacc as bacc
import concourse.mybir as mybir
import concourse.tile as tile
from concourse.bass_utils import run_bass_kernel_spmd

HEADS = 16
D = 1024
DH = 64
S = 4096
N_CORES = 8
SQ = S // N_CORES          # query rows owned per core (output sharding)
HPC = HEADS // N_CORES     # heads per core
BF16 = mybir.dt.bfloat16
F32 = mybir.dt.float32

NEG_BIG = -3840.0          # exp(-3840) == 0.0 exactly in fp32
EXP_GROUP = 2              # k-chunks (PSUM banks) per exp activation op
I16 = mybir.dt.int16
S_BUFS = 3
A_BUFS = 3
EXPJ = 0
QIN_SPLITS = 2
N_WARM = 111
KV_INTERLEAVE = 1
CTX_BUFS = 2
CEXP_BUFS = 5
CMISC_BUFS = 3
LOG2E = 1.4426950408889634
A16 = 0.125 * 128.0 * LOG2E   # DVE exp trick: bf16 = i16(x*A16 + B16)
B16 = 127.0 * 128.0


def _bf16(x):
    return np.ascontiguousarray(x.astype(ml_dtypes.bfloat16))


def build_program(n_pad, kc_real, kc_mixed, repeat=1, ablate=()):
    """Build the 8-core SPMD program.

    n_pad: padded compacted key count (multiple of 512).
    kc_real: number of leading k-chunks (of 128) with no padding.
    kc_mixed: 1 if a chunk straddles n (it gets a per-partition bias column
    on its exp); chunks past kc_real + kc_mixed are fully padded and get a
    constant NEG_BIG bias.
    """
    KC = n_pad // 128
    kblocks = []
    _b0 = 0
    while _b0 < n_pad:
        kblocks.append((_b0, min(512, n_pad - _b0)))
        _b0 += 512
    QC = S // 512            # query column groups (whole sequence)
    nc = bacc.Bacc("TRN2", target_bir_lowering=False, debug=False,
                   num_devices=N_CORES)

    # ---- I/O ----  (all bf16 unless noted; feature-major activations)
    qT = nc.dram_tensor("qT", [D, S], BF16, kind="ExternalInput")
    kcT = nc.dram_tensor("kcT", [D, n_pad], BF16, kind="ExternalInput")
    vcT = nc.dram_tensor("vcT", [D, n_pad], BF16, kind="ExternalInput")
    wqT = nc.dram_tensor("wqT", [D, HPC * DH], BF16, kind="ExternalInput")
    wkT = nc.dram_tensor("wkT", [D, HPC * DH], BF16, kind="ExternalInput")
    wvT = nc.dram_tensor("wvT", [D, HPC * DH], BF16, kind="ExternalInput")
    woT = nc.dram_tensor("woT", [D, D], BF16, kind="ExternalInput")
    bq_m = nc.dram_tensor("bq_m", [128, 1], F32, kind="ExternalInput")
    bk_m = nc.dram_tensor("bk_m", [128, 1], F32, kind="ExternalInput")
    bv_m = nc.dram_tensor("bv_m", [1, HPC * DH], BF16, kind="ExternalInput")
    bo_r = nc.dram_tensor("bo_r", [1, D], BF16, kind="ExternalInput")
    # per k-chunk exp bias column (0 for real keys, NEG_BIG for padding)
    pbias = nc.dram_tensor("pbias", [128, KC], F32, kind="ExternalInput")
    out = nc.dram_tensor("out", [SQ, D], F32, kind="ExternalOutput")

    with tile.TileContext(nc) as tc:
        for _rep in range(repeat):
            with (
                tc.tile_pool(name="dram", bufs=1, space="DRAM") as dram,
                tc.tile_pool(name="consts", bufs=1) as consts,
                tc.tile_pool(name="persist", bufs=1) as persist,
            ):
                # per-head A2A buffers: shard q-block -> [64 feats, 512 q]
                a2a_in = [dram.tile([N_CORES, 64, 512], BF16, name=f"a2i{j}")
                          for j in range(HPC)]
                a2a_out = [dram.tile([N_CORES, 64, 512], BF16, name=f"a2o{j}")
                           for j in range(HPC)]

                ones_bf = consts.tile([1, 128], BF16)
                nc.vector.memset(ones_bf[:], 1.0)
                ones_f = consts.tile([1, 64], F32)
                nc.vector.memset(ones_f[:], 1.0)
                bq_sb = consts.tile([128, 1], F32)
                nc.sync.dma_start(bq_sb[:], bq_m[:])
                bk_sb = consts.tile([128, 1], F32)
                nc.sync.dma_start(bk_sb[:], bk_m[:])
                bv_sb = consts.tile([1, HPC * DH], BF16)
                nc.sync.dma_start(bv_sb[:], bv_m[:])
                bo_sb = consts.tile([1, D], BF16)
                nc.sync.dma_start(bo_sb[:], bo_r[:])
                pb_sb = consts.tile([128, KC], F32)
                nc.sync.dma_start(pb_sb[:], pbias[:])

                kT_all = persist.tile([128, n_pad], BF16)
                wq_sb = persist.tile([128, 8, HPC * DH], BF16)
                q0_in = persist.tile([128, 8, 512], BF16)
                # v layout: [n-part, k-chunk, head, DH+1]; col DH == ones
                v_all = persist.tile([128, KC, HPC, DH + 1], BF16)
                q_pair = persist.tile([128, QC, 512], BF16)

                # ---------- Phase A: K/V projection (own 2 heads) ----------
                with (
                    tc.tile_pool(name="a_w", bufs=1) as a_w,
                    tc.tile_pool(name="a_in", bufs=1) as a_in,
                    tc.tile_pool(name="a_ps", bufs=A_BUFS, space="PSUM") as a_ps,
                ):
                    wk_sb = a_w.tile([128, 8, HPC * DH], BF16)
                    wv_sb = a_w.tile([128, 8, HPC * DH], BF16)
                    nc.sync.dma_start(wk_sb[:],
                                      wkT.rearrange("(c p) m -> p c m",
                                                    p=128))
                    nc.scalar.dma_start(wv_sb[:],
                                        wvT.rearrange("(c p) m -> p c m",
                                                      p=128))
                    nc.scalar.dma_start(wq_sb[:],
                                        wqT.rearrange("(c p) m -> p c m",
                                                      p=128))
                    nc.sync.dma_start(q0_in[:],
                                      qT[:, 0:512].rearrange(
                                          "(c p) m -> p c m", p=128))
                    nc.vector.memset(v_all[:, :, :, DH:DH + 1], 1.0)

                    # one fat contiguous DMA per 128-row chunk
                    kin = a_in.tile([128, 8, n_pad], BF16)
                    vin = a_in.tile([128, 8, n_pad], BF16)
                    if KV_INTERLEAVE:
                        nh = (n_pad // 2 + 511) // 512 * 512
                        nh = min(nh, n_pad)
                        nc.sync.dma_start(
                            kin[:, :, 0:nh],
                            kcT[:, 0:nh].rearrange("(c p) n -> p c n", p=128))
                        nc.scalar.dma_start(
                            vin[:, :, 0:nh],
                            vcT[:, 0:nh].rearrange("(c p) n -> p c n", p=128))
                        if nh < n_pad:
                            nc.sync.dma_start(
                                kin[:, :, nh:],
                                kcT[:, nh:].rearrange("(c p) n -> p c n",
                                                      p=128))
                            nc.scalar.dma_start(
                                vin[:, :, nh:],
                                vcT[:, nh:].rearrange("(c p) n -> p c n",
                                                      p=128))
                    else:
                        nc.sync.dma_start(kin[:],
                                          kcT.rearrange("(c p) n -> p c n",
                                                        p=128))
                        nc.scalar.dma_start(vin[:],
                                            vcT.rearrange("(c p) n -> p c n",
                                                          p=128))

                    ps_q0 = a_ps.tile([128, 512], F32, tag="psk")
                    for c in range(8):
                        nc.tensor.matmul(ps_q0[:], wq_sb[:, c, :],
                                         q0_in[:, c, :],
                                         start=(c == 0), stop=(c == 7))
                    nc.vector.tensor_scalar_add(q_pair[:, 0, :], ps_q0[:],
                                                bq_sb[:])

                    for (b0, bw) in kblocks:
                        ns = slice(b0, b0 + bw)
                        ps_k = a_ps.tile([128, 512], F32, tag="psk")
                        for c in range(8):
                            nc.tensor.matmul(ps_k[:, 0:bw], wk_sb[:, c, :],
                                             kin[:, c, ns],
                                             start=(c == 0), stop=(c == 7))
                        nc.vector.tensor_scalar_add(kT_all[:, ns],
                                                    ps_k[:, 0:bw], bk_sb[:])
                    for kc in range(KC):
                        ks = slice(kc * 128, (kc + 1) * 128)
                        ps_v = a_ps.tile([128, HPC * DH], F32, tag="psv")
                        for c in range(8):
                            nc.tensor.matmul(
                                ps_v[:], vin[:, c, ks],
                                wv_sb[:, c, :], start=(c == 0), stop=False)
                        ps_v_done = nc.tensor.matmul(
                            ps_v[:], ones_bf[:, :128],
                            bv_sb[:], start=False, stop=True)
                        eng = nc.vector if kc % 2 else nc.scalar
                        if kc % 2:
                            nc.vector.tensor_copy(
                                v_all[:, kc, :, 0:DH],
                                ps_v[:].rearrange("p (j d) -> p j d", j=HPC))
                        else:
                            nc.scalar.copy(
                                v_all[:, kc, :, 0:DH],
                                ps_v[:].rearrange("p (j d) -> p j d", j=HPC))

                # ---------- Phase B folded into C: q blocks on demand --
                # behind kin/vin on the two HWDGE queues so the big qT load
                # cannot delay phase A's inputs
                qin = persist.tile([128, 8, S - 512], BF16)
                for qh in range(QIN_SPLITS):
                    w = (S - 512) // QIN_SPLITS
                    a, b = qh * w, (qh + 1) * w
                    nc.scalar.dma_start(
                        qin[:, :, a:b],
                        qT[:, 512 + a:512 + b].rearrange(
                            "(c p) m -> p c m", p=128))

                # ---------- Phase C: attention for own 2 heads ----------
                # wo is loaded early so phase D's weights are resident
                wo_sb2 = persist.tile([128, N_CORES, D], BF16)
                nc.gpsimd.dma_start(wo_sb2[:],
                                    woT.rearrange("(c p) m -> p c m",
                                                  p=128))
                with (
                    tc.tile_pool(name="c_exp", bufs=CEXP_BUFS) as c_exp,
                    tc.tile_pool(name="c_misc", bufs=CMISC_BUFS) as c_misc,
                    tc.tile_pool(name="c_ps_s", bufs=S_BUFS, space="PSUM") as c_ps_s,
                    tc.tile_pool(name="c_ps_c", bufs=CTX_BUFS, space="PSUM") as c_ps_c,
                ):
                    for j in range(HPC):
                        pj = slice(64 * j, 64 * (j + 1))
                        for qc in range(QC):
                            if j == 0 and qc + 1 < QC:
                                qs = slice(qc * 512, (qc + 1) * 512)
                                ps_q = c_ps_s.tile([128, EXP_GROUP, 512], F32,
                                                   tag="s", name=f"psq{qc}")
                                for c in range(8):
                                    nc.tensor.matmul(
                                        ps_q[:, 0, :], wq_sb[:, c, :],
                                        qin[:, c, qs],
                                        start=(c == 0), stop=(c == 7))
                                nc.vector.tensor_scalar_add(
                                    q_pair[:, qc + 1, :], ps_q[:, 0, :],
                                    bq_sb[:])
                            ps_ctx = c_ps_c.tile([128, 512], F32, tag="ctx")
                            rhs_q = q_pair[pj, qc, :]
                            c0 = 0
                            gi = 0
                            while c0 < KC:
                                gn = min(EXP_GROUP, KC - c0)
                                ps_s = c_ps_s.tile([128, EXP_GROUP, 512], F32,
                                                   tag="s")
                                for cc in range(gn):
                                    lc = c0 + cc
                                    nc.tensor.matmul(
                                        ps_s[:, cc, :],
                                        kT_all[pj,
                                               lc * 128:(lc + 1) * 128],
                                        rhs_q, start=True, stop=True,
                                        tile_position=(64 * j, 0))
                                exp_sb = c_exp.tile([128, EXP_GROUP, 512],
                                                    BF16, tag="e")
                                clean = c0 + gn <= kc_real
                                if clean and (gi + qc + EXPJ * j) % 2 == 0:
                                    # exp2 bit-trick: bf16 == i16(x*A16+B16)
                                    nc.vector.tensor_scalar(
                                        exp_sb[:, 0:gn, :].bitcast(I16),
                                        ps_s[:, 0:gn, :], A16, B16,
                                        mybir.AluOpType.mult,
                                        mybir.AluOpType.add)
                                elif clean:
                                    nc.scalar.activation(
                                        exp_sb[:, 0:gn, :], ps_s[:, 0:gn, :],
                                        mybir.ActivationFunctionType.Exp,
                                        bias=0.0, scale=0.125)
                                else:
                                    for cc in range(gn):
                                        nc.scalar.activation(
                                            exp_sb[:, cc, :], ps_s[:, cc, :],
                                            mybir.ActivationFunctionType.Exp,
                                            bias=pb_sb[:, c0 + cc:c0 + cc + 1],
                                            scale=0.125)
                                for cc in range(gn):
                                    lc = c0 + cc
                                    nc.tensor.matmul(
                                        ps_ctx[0:DH + 1, :],
                                        v_all[:, lc, j, :],
                                        exp_sb[:, cc, :],
                                        start=(lc == 0),
                                        stop=(lc == KC - 1))
                                c0 += gn
                                gi += 1

                            recip = c_misc.tile([1, 512], BF16, tag="recip")
                            with nc.allow_low_precision(reason="1/d bf16"):
                                nc.vector.reciprocal(recip[:],
                                                     ps_ctx[DH:DH + 1, :])
                            nc.tensor.matmul(
                                ps_ctx[64:128, :],
                                ones_bf[0:1, 0:64], recip[:],
                                start=True, stop=True,
                                tile_position=(0, 64),
                                skip_group_check=True)
                            rec_bc = c_misc.tile([64, 512], F32, tag="rbc")
                            nc.scalar.copy(rec_bc[:], ps_ctx[64:128, :])
                            ctx_sb = c_misc.tile([64, 512], BF16, tag="ctxs")
                            nc.vector.tensor_mul(ctx_sb[:], ps_ctx[0:64, :],
                                                 rec_bc[:])
                            nc.sync.dma_start(a2a_in[j][qc], ctx_sb[:])

                        if "cclocal" in ablate:
                            nc.sync.dma_start(a2a_out[j][:], a2a_in[j][:])
                        else:
                            nc.gpsimd.collective_compute(
                                "AllToAll", mybir.AluOpType.bypass,
                                replica_groups=[list(range(N_CORES))],
                                ins=[a2a_in[j].opt()],
                                outs=[a2a_out[j].opt()])

                # ---------- Phase D: output projection (own 512 rows) ----------
                if "noD" in ablate:
                    continue
                with (
                    tc.tile_pool(name="d_w", bufs=1) as d_w,
                    tc.tile_pool(name="d_out", bufs=3) as d_out,
                    tc.tile_pool(name="d_ps", bufs=8, space="PSUM") as d_ps,
                ):
                    # heads of equal j stacked in pairs on partitions (K=128);
                    # D0 (j=0 pairs) depends only on A2A_0 and overlaps A2A_1.
                    # The 8 (qc, eh) PSUM tiles stay open across both j passes.
                    ctx_p = [d_w.tile([128, 4, 512], BF16, name=f"cxp{j}")
                             for j in range(HPC)]
                    zz = d_w.tile([128, 128], BF16)
                    nc.vector.memset(zz[:], 0.0)
                    ps_os = {}
                    def warm_keeper():
                        for i in range(N_WARM):
                            nc.tensor.matmul(
                                ps_os[(0, 0)][:], zz[:],
                                qin[:, i % 8, 0:512],
                                start=False, stop=False)
                    for j in range(HPC):
                        if j == 1:
                            warm_keeper()
                        ev = a2a_out[j].rearrange("(a two) p q -> a two p q",
                                                  two=2)
                        nc.sync.dma_start(
                            ctx_p[j][0:64, :, :],
                            ev[:, 0].rearrange("a p q -> p a q"))
                        nc.sync.dma_start(
                            ctx_p[j][64:128, :, :],
                            ev[:, 1].rearrange("a p q -> p a q"))
                        for qc in range(SQ // 128):
                            for eh in range(2):
                                es = slice(eh * 512, (eh + 1) * 512)
                                if j == 0:
                                    ps_o = d_ps.tile([128, 512], F32,
                                                     tag="pso",
                                                     name=f"po{qc}{eh}")
                                    ps_os[(qc, eh)] = ps_o
                                else:
                                    ps_o = ps_os[(qc, eh)]
                                for a in range(4):
                                    nc.tensor.matmul(
                                        ps_o[:],
                                        ctx_p[j][:, a,
                                                 qc * 128:(qc + 1) * 128],
                                        wo_sb2[:, 4 * j + a, es],
                                        start=(j == 0 and a == 0),
                                        stop=(j == 1 and a == 3))
                                if j == 0:
                                    nc.tensor.matmul(
                                        ps_o[:], ones_bf[:, 0:128],
                                        bo_sb[:, es], start=False, stop=False)
                                else:
                                    o_sb = d_out.tile([128, 512], F32,
                                                      tag="osb")
                                    if (qc + eh) % 2:
                                        nc.vector.tensor_copy(o_sb[:],
                                                              ps_o[:])
                                    else:
                                        nc.scalar.copy(o_sb[:], ps_o[:])
                                    nc.sync.dma_start(
                                        out[qc * 128:(qc + 1) * 128, es],
                                        o_sb[:])

    nc.compile()
    return nc


def prepare(query, key, value, mask, Wq, bq, Wk, bk, Wv, bv, Wo, bo):
    """Host-side sharding/preprocessing + program build. Returns the compiled
    Bass program and the per-core input maps."""
    query = np.asarray(query)
    key = np.asarray(key)
    value = np.asarray(value)
    mask = np.asarray(mask)
    Wq, bq = np.asarray(Wq), np.asarray(bq)
    Wk, bk = np.asarray(Wk), np.asarray(bk)
    Wv, bv = np.asarray(Wv), np.asarray(bv)
    Wo, bo = np.asarray(Wo), np.asarray(bo)

    idx = np.nonzero(mask.reshape(-1) != 0)[0]
    n = int(idx.size)
    n_pad = max(512, ((n + 127) // 128) * 128)
    KC = n_pad // 128
    kc_real = n // 128
    kc_mixed = 1 if n % 128 else 0

    key_c = np.zeros((n_pad, D), np.float32)
    val_c = np.zeros((n_pad, D), np.float32)
    key_c[:n] = key[0, idx, :]
    val_c[:n] = value[0, idx, :]

    kcT_np = _bf16(key_c.T)
    vcT_np = _bf16(val_c.T)
    qT_np = _bf16(query[0].T)
    wqT_np = _bf16(Wq.T)
    wkT_np = _bf16(Wk.T)
    wvT_np = _bf16(Wv.T)
    woT_r = Wo.T
    slots = []
    for j in range(HPC):
        for a in range(4):
            hA, hB = 4 * a + j, 4 * a + 2 + j
            slots.append(woT_r[64 * hA:64 * hA + 64, :])
            slots.append(woT_r[64 * hB:64 * hB + 64, :])
    woT_np = _bf16(np.concatenate(slots, axis=0))
    bo_r_np = _bf16(bo.reshape(1, D))

    pb_np = np.zeros((128, KC), np.float32)
    flat = np.full(n_pad, NEG_BIG, np.float32)
    flat[:n] = 0.0
    pb_np[:] = flat.reshape(KC, 128).T

    nc = build_program(n_pad, kc_real, kc_mixed)

    in_maps = []
    for m in range(N_CORES):
        sl = slice(m * 128, (m + 1) * 128)
        in_maps.append({
            "qT": qT_np,
            "kcT": kcT_np,
            "vcT": vcT_np,
            "wqT": np.ascontiguousarray(wqT_np[:, sl]),
            "wkT": np.ascontiguousarray(wkT_np[:, sl]),
            "wvT": np.ascontiguousarray(wvT_np[:, sl]),
            "woT": woT_np,
            "bq_m": np.ascontiguousarray(
                bq[sl].reshape(128, 1).astype(np.float32)),
            "bk_m": np.ascontiguousarray(
                bk[sl].reshape(128, 1).astype(np.float32)),
            "bv_m": _bf16(bv[sl].reshape(1, 128)),
            "bo_r": bo_r_np,
            "pbias": pb_np,
        })

    return {"nc": nc, "in_maps": in_maps, "n": n, "n_pad": n_pad}


def kernel(query, key, value, mask, Wq, bq, Wk, bk, Wv, bv, Wo, bo,
           _trace=False, _result_box=None):
    prep = prepare(query, key, value, mask, Wq, bq, Wk, bk, Wv, bv, Wo, bo)
    res = run_bass_kernel_spmd(prep["nc"], prep["in_maps"],
                               list(range(N_CORES)), trace=_trace)
    if _result_box is not None:
        _result_box.append(res)

    out = np.concatenate([res.results[m]["out"] for m in range(N_CORES)],
                         axis=0)
    return out.reshape(1, S, D).astype(np.float32)



# revision 6
# speedup vs baseline: 1.0302x; 1.0302x over previous
"""Multi-head attention (B=1, S=4096, D=1024, H=16) on 8 TRN2 NeuronCores.

Strategy (head-sharded attention + AllToAll context exchange):
  - Host: compact K/V to the unmasked key positions (mask==0 keys contribute
    exactly 0 to softmax numerator and denominator, since the reference's
    -1e9 masking underflows exp to 0.0), transpose activations/weights to
    feature-major, cast matmul operands to bf16.
  - Phase A: core m computes K^T and V projections for its 2 heads over all
    compacted positions; results stay in SBUF (no gather needed).
  - Phase B: Q projection for the same 2 heads over ALL 4096 queries.
  - Phase C: attention for the 2 heads x 4096 queries: scores^T =
    K^T-chunk.T @ Q^T in PSUM ([k,q] layout, per-head via matmul
    tile_position row groups), exp on ScalarE straight out of PSUM (padding
    bias folded into the per-partition activation bias), P@V with a
    ones-augmented V (row 64 = softmax denominators), reciprocal + K=1
    broadcast matmul + multiply to normalize. Per-head context goes to DRAM
    sliced by query block.
  - AllToAll (one per head, 2 MiB, the first overlaps the second head's
    compute) converts head-sharding to query-sharding: afterwards core m
    holds all 16 heads' context for its own 512 queries.
  - Phase D: output projection of the core's 512 rows. The host just
    concatenates the 8 row-slices.
"""

import numpy as np
import ml_dtypes

import concourse.bacc as bacc
import concourse.mybir as mybir
import concourse.tile as tile
from concourse.bass_utils import run_bass_kernel_spmd

HEADS = 16
D = 1024
DH = 64
S = 4096
N_CORES = 8
SQ = S // N_CORES          # query rows owned per core (output sharding)
HPC = HEADS // N_CORES     # heads per core
BF16 = mybir.dt.bfloat16
F32 = mybir.dt.float32

NEG_BIG = -3840.0          # exp(-3840) == 0.0 exactly in fp32
EXP_GROUP = 2              # k-chunks (PSUM banks) per exp activation op
I16 = mybir.dt.int16
S_BUFS = 3
A_BUFS = 3
EXPJ = 0
QIN_SPLITS = 2
N_WARM = 111
N_WARM_A = 38
WO_SPLITS = 1
WO_DELAY = 0
KV_INTERLEAVE = 1
CTX_BUFS = 2
CEXP_BUFS = 5
CMISC_BUFS = 3
LOG2E = 1.4426950408889634
A16 = 0.125 * 128.0 * LOG2E   # DVE exp trick: bf16 = i16(x*A16 + B16)
B16 = 127.0 * 128.0


def _bf16(x):
    return np.ascontiguousarray(x.astype(ml_dtypes.bfloat16))


def build_program(n_pad, kc_real, kc_mixed, repeat=1, ablate=()):
    """Build the 8-core SPMD program.

    n_pad: padded compacted key count (multiple of 512).
    kc_real: number of leading k-chunks (of 128) with no padding.
    kc_mixed: 1 if a chunk straddles n (it gets a per-partition bias column
    on its exp); chunks past kc_real + kc_mixed are fully padded and get a
    constant NEG_BIG bias.
    """
    KC = n_pad // 128
    kblocks = []
    _b0 = 0
    while _b0 < n_pad:
        kblocks.append((_b0, min(512, n_pad - _b0)))
        _b0 += 512
    QC = S // 512            # query column groups (whole sequence)
    nc = bacc.Bacc("TRN2", target_bir_lowering=False, debug=False,
                   num_devices=N_CORES)

    # ---- I/O ----  (all bf16 unless noted; feature-major activations)
    qT = nc.dram_tensor("qT", [D, S], BF16, kind="ExternalInput")
    kcT = nc.dram_tensor("kcT", [D, n_pad], BF16, kind="ExternalInput")
    vcT = nc.dram_tensor("vcT", [D, n_pad], BF16, kind="ExternalInput")
    wqT = nc.dram_tensor("wqT", [D, HPC * DH], BF16, kind="ExternalInput")
    wkT = nc.dram_tensor("wkT", [D, HPC * DH], BF16, kind="ExternalInput")
    wvT = nc.dram_tensor("wvT", [D, HPC * DH], BF16, kind="ExternalInput")
    woT = nc.dram_tensor("woT", [D, D], BF16, kind="ExternalInput")
    bq_m = nc.dram_tensor("bq_m", [128, 1], F32, kind="ExternalInput")
    bk_m = nc.dram_tensor("bk_m", [128, 1], F32, kind="ExternalInput")
    bv_m = nc.dram_tensor("bv_m", [1, HPC * DH], BF16, kind="ExternalInput")
    bo_r = nc.dram_tensor("bo_r", [1, D], BF16, kind="ExternalInput")
    # per k-chunk exp bias column (0 for real keys, NEG_BIG for padding)
    pbias = nc.dram_tensor("pbias", [128, KC], F32, kind="ExternalInput")
    out = nc.dram_tensor("out", [SQ, D], F32, kind="ExternalOutput")

    with tile.TileContext(nc) as tc:
        for _rep in range(repeat):
            with (
                tc.tile_pool(name="dram", bufs=1, space="DRAM") as dram,
                tc.tile_pool(name="consts", bufs=1) as consts,
                tc.tile_pool(name="persist", bufs=1) as persist,
            ):
                # per-head A2A buffers: shard q-block -> [64 feats, 512 q]
                a2a_in = [dram.tile([N_CORES, 64, 512], BF16, name=f"a2i{j}")
                          for j in range(HPC)]
                a2a_out = [dram.tile([N_CORES, 64, 512], BF16, name=f"a2o{j}")
                           for j in range(HPC)]

                ones_bf = consts.tile([1, 128], BF16)
                nc.vector.memset(ones_bf[:], 1.0)
                ones_f = consts.tile([1, 64], F32)
                nc.vector.memset(ones_f[:], 1.0)
                bq_sb = consts.tile([128, 1], F32)
                nc.sync.dma_start(bq_sb[:], bq_m[:])
                bk_sb = consts.tile([128, 1], F32)
                nc.sync.dma_start(bk_sb[:], bk_m[:])
                bv_sb = consts.tile([1, HPC * DH], BF16)
                nc.sync.dma_start(bv_sb[:], bv_m[:])
                bo_sb = consts.tile([1, D], BF16)
                nc.sync.dma_start(bo_sb[:], bo_r[:])
                pb_sb = consts.tile([128, KC], F32)
                nc.sync.dma_start(pb_sb[:], pbias[:])

                kT_all = persist.tile([128, n_pad], BF16)
                wq_sb = persist.tile([128, 8, HPC * DH], BF16)
                q0_in = persist.tile([128, 8, 512], BF16)
                # v layout: [n-part, k-chunk, head, DH+1]; col DH == ones
                v_all = persist.tile([128, KC, HPC, DH + 1], BF16)
                q_pair = persist.tile([128, QC, 512], BF16)

                # ---------- Phase A: K/V projection (own 2 heads) ----------
                with (
                    tc.tile_pool(name="a_w", bufs=1) as a_w,
                    tc.tile_pool(name="a_in", bufs=1) as a_in,
                    tc.tile_pool(name="a_ps", bufs=A_BUFS, space="PSUM") as a_ps,
                    tc.tile_pool(name="a_psw", bufs=1, space="PSUM") as a_psw,
                ):
                    wk_sb = a_w.tile([128, 8, HPC * DH], BF16)
                    wv_sb = a_w.tile([128, 8, HPC * DH], BF16)
                    nc.sync.dma_start(wk_sb[:],
                                      wkT.rearrange("(c p) m -> p c m",
                                                    p=128))
                    nc.scalar.dma_start(wv_sb[:],
                                        wvT.rearrange("(c p) m -> p c m",
                                                      p=128))
                    nc.scalar.dma_start(wq_sb[:],
                                        wqT.rearrange("(c p) m -> p c m",
                                                      p=128))
                    nc.sync.dma_start(q0_in[:],
                                      qT[:, 0:512].rearrange(
                                          "(c p) m -> p c m", p=128))
                    nc.vector.memset(v_all[:, :, :, DH:DH + 1], 1.0)

                    # one fat contiguous DMA per 128-row chunk
                    kin = a_in.tile([128, 8, n_pad], BF16)
                    vin = a_in.tile([128, 8, n_pad], BF16)
                    if KV_INTERLEAVE:
                        nh = (n_pad // 2 + 511) // 512 * 512
                        nh = min(nh, n_pad)
                        nc.sync.dma_start(
                            kin[:, :, 0:nh],
                            kcT[:, 0:nh].rearrange("(c p) n -> p c n", p=128))
                        nc.scalar.dma_start(
                            vin[:, :, 0:nh],
                            vcT[:, 0:nh].rearrange("(c p) n -> p c n", p=128))
                        if nh < n_pad:
                            nc.sync.dma_start(
                                kin[:, :, nh:],
                                kcT[:, nh:].rearrange("(c p) n -> p c n",
                                                      p=128))
                            nc.scalar.dma_start(
                                vin[:, :, nh:],
                                vcT[:, nh:].rearrange("(c p) n -> p c n",
                                                      p=128))
                    else:
                        nc.sync.dma_start(kin[:],
                                          kcT.rearrange("(c p) n -> p c n",
                                                        p=128))
                        nc.scalar.dma_start(vin[:],
                                            vcT.rearrange("(c p) n -> p c n",
                                                          p=128))

                    if N_WARM_A:
                        wmov = a_w.tile([128, 512], BF16)
                        nc.vector.memset(wmov[:], 0.0)
                        wst = a_w.tile([128, 128], BF16)
                        nc.vector.memset(wst[:], 0.0)
                        ps_w = a_psw.tile([128, 512], F32, tag="pw")
                        for i in range(N_WARM_A):
                            nc.tensor.matmul(ps_w[:], wst[:], wmov[:],
                                             start=(i == 0),
                                             stop=(i == N_WARM_A - 1))
                    ps_q0 = a_ps.tile([128, 512], F32, tag="psk")
                    for c in range(8):
                        nc.tensor.matmul(ps_q0[:], wq_sb[:, c, :],
                                         q0_in[:, c, :],
                                         start=(c == 0), stop=(c == 7))
                    nc.vector.tensor_scalar_add(q_pair[:, 0, :], ps_q0[:],
                                                bq_sb[:])

                    for (b0, bw) in kblocks:
                        ns = slice(b0, b0 + bw)
                        ps_k = a_ps.tile([128, 512], F32, tag="psk")
                        for c in range(8):
                            nc.tensor.matmul(ps_k[:, 0:bw], wk_sb[:, c, :],
                                             kin[:, c, ns],
                                             start=(c == 0), stop=(c == 7))
                        nc.vector.tensor_scalar_add(kT_all[:, ns],
                                                    ps_k[:, 0:bw], bk_sb[:])
                    for kc in range(KC):
                        ks = slice(kc * 128, (kc + 1) * 128)
                        ps_v = a_ps.tile([128, HPC * DH], F32, tag="psv")
                        for c in range(8):
                            nc.tensor.matmul(
                                ps_v[:], vin[:, c, ks],
                                wv_sb[:, c, :], start=(c == 0), stop=False)
                        ps_v_done = nc.tensor.matmul(
                            ps_v[:], ones_bf[:, :128],
                            bv_sb[:], start=False, stop=True)
                        eng = nc.vector if kc % 2 else nc.scalar
                        if kc % 2:
                            nc.vector.tensor_copy(
                                v_all[:, kc, :, 0:DH],
                                ps_v[:].rearrange("p (j d) -> p j d", j=HPC))
                        else:
                            nc.scalar.copy(
                                v_all[:, kc, :, 0:DH],
                                ps_v[:].rearrange("p (j d) -> p j d", j=HPC))

                # ---------- Phase B folded into C: q blocks on demand --
                # behind kin/vin on the two HWDGE queues so the big qT load
                # cannot delay phase A's inputs
                qin = persist.tile([128, 8, S - 512], BF16)
                for qh in range(QIN_SPLITS):
                    w = (S - 512) // QIN_SPLITS
                    a, b = qh * w, (qh + 1) * w
                    nc.scalar.dma_start(
                        qin[:, :, a:b],
                        qT[:, 512 + a:512 + b].rearrange(
                            "(c p) m -> p c m", p=128))

                # ---------- Phase C: attention for own 2 heads ----------
                # wo is loaded early so phase D's weights are resident
                wo_sb2 = persist.tile([128, N_CORES, D], BF16)
                if WO_DELAY:
                    # hold the gpsimd queue until kin lands so the wo load
                    # cannot occupy the DMA device ahead of phase-A inputs
                    wo_gate = persist.tile([1, 8], BF16)
                    nc.gpsimd.tensor_copy(wo_gate[:], qin[0:1, :, 0:1])
                for wh in range(WO_SPLITS):
                    w = D // WO_SPLITS
                    a, b = wh * w, (wh + 1) * w
                    nc.gpsimd.dma_start(
                        wo_sb2[:, :, a:b],
                        woT[:, a:b].rearrange("(c p) m -> p c m", p=128))
                with (
                    tc.tile_pool(name="c_exp", bufs=CEXP_BUFS) as c_exp,
                    tc.tile_pool(name="c_misc", bufs=CMISC_BUFS) as c_misc,
                    tc.tile_pool(name="c_ps_s", bufs=S_BUFS, space="PSUM") as c_ps_s,
                    tc.tile_pool(name="c_ps_c", bufs=CTX_BUFS, space="PSUM") as c_ps_c,
                ):
                    for j in range(HPC):
                        pj = slice(64 * j, 64 * (j + 1))
                        for qc in range(QC):
                            if j == 0 and qc + 1 < QC:
                                qs = slice(qc * 512, (qc + 1) * 512)
                                ps_q = c_ps_s.tile([128, EXP_GROUP, 512], F32,
                                                   tag="s", name=f"psq{qc}")
                                for c in range(8):
                                    nc.tensor.matmul(
                                        ps_q[:, 0, :], wq_sb[:, c, :],
                                        qin[:, c, qs],
                                        start=(c == 0), stop=(c == 7))
                                nc.vector.tensor_scalar_add(
                                    q_pair[:, qc + 1, :], ps_q[:, 0, :],
                                    bq_sb[:])
                            ps_ctx = c_ps_c.tile([128, 512], F32, tag="ctx")
                            rhs_q = q_pair[pj, qc, :]
                            c0 = 0
                            gi = 0
                            while c0 < KC:
                                gn = min(EXP_GROUP, KC - c0)
                                ps_s = c_ps_s.tile([128, EXP_GROUP, 512], F32,
                                                   tag="s")
                                for cc in range(gn):
                                    lc = c0 + cc
                                    nc.tensor.matmul(
                                        ps_s[:, cc, :],
                                        kT_all[pj,
                                               lc * 128:(lc + 1) * 128],
                                        rhs_q, start=True, stop=True,
                                        tile_position=(64 * j, 0))
                                exp_sb = c_exp.tile([128, EXP_GROUP, 512],
                                                    BF16, tag="e")
                                clean = c0 + gn <= kc_real
                                if clean and (gi + qc + EXPJ * j) % 2 == 0:
                                    # exp2 bit-trick: bf16 == i16(x*A16+B16)
                                    nc.vector.tensor_scalar(
                                        exp_sb[:, 0:gn, :].bitcast(I16),
                                        ps_s[:, 0:gn, :], A16, B16,
                                        mybir.AluOpType.mult,
                                        mybir.AluOpType.add)
                                elif clean:
                                    nc.scalar.activation(
                                        exp_sb[:, 0:gn, :], ps_s[:, 0:gn, :],
                                        mybir.ActivationFunctionType.Exp,
                                        bias=0.0, scale=0.125)
                                else:
                                    for cc in range(gn):
                                        nc.scalar.activation(
                                            exp_sb[:, cc, :], ps_s[:, cc, :],
                                            mybir.ActivationFunctionType.Exp,
                                            bias=pb_sb[:, c0 + cc:c0 + cc + 1],
                                            scale=0.125)
                                for cc in range(gn):
                                    lc = c0 + cc
                                    nc.tensor.matmul(
                                        ps_ctx[0:DH + 1, :],
                                        v_all[:, lc, j, :],
                                        exp_sb[:, cc, :],
                                        start=(lc == 0),
                                        stop=(lc == KC - 1))
                                c0 += gn
                                gi += 1

                            recip = c_misc.tile([1, 512], BF16, tag="recip")
                            with nc.allow_low_precision(reason="1/d bf16"):
                                nc.vector.reciprocal(recip[:],
                                                     ps_ctx[DH:DH + 1, :])
                            nc.tensor.matmul(
                                ps_ctx[64:128, :],
                                ones_bf[0:1, 0:64], recip[:],
                                start=True, stop=True,
                                tile_position=(0, 64),
                                skip_group_check=True)
                            rec_bc = c_misc.tile([64, 512], F32, tag="rbc")
                            nc.scalar.copy(rec_bc[:], ps_ctx[64:128, :])
                            ctx_sb = c_misc.tile([64, 512], BF16, tag="ctxs")
                            nc.vector.tensor_mul(ctx_sb[:], ps_ctx[0:64, :],
                                                 rec_bc[:])
                            nc.sync.dma_start(a2a_in[j][qc], ctx_sb[:])

                        if "cclocal" in ablate:
                            nc.sync.dma_start(a2a_out[j][:], a2a_in[j][:])
                        else:
                            nc.gpsimd.collective_compute(
                                "AllToAll", mybir.AluOpType.bypass,
                                replica_groups=[list(range(N_CORES))],
                                ins=[a2a_in[j].opt()],
                                outs=[a2a_out[j].opt()])

                # ---------- Phase D: output projection (own 512 rows) ----------
                if "noD" in ablate:
                    continue
                with (
                    tc.tile_pool(name="d_w", bufs=1) as d_w,
                    tc.tile_pool(name="d_out", bufs=3) as d_out,
                    tc.tile_pool(name="d_ps", bufs=8, space="PSUM") as d_ps,
                ):
                    # heads of equal j stacked in pairs on partitions (K=128);
                    # D0 (j=0 pairs) depends only on A2A_0 and overlaps A2A_1.
                    # The 8 (qc, eh) PSUM tiles stay open across both j passes.
                    ctx_p = [d_w.tile([128, 4, 512], BF16, name=f"cxp{j}")
                             for j in range(HPC)]
                    zz = d_w.tile([128, 128], BF16)
                    nc.vector.memset(zz[:], 0.0)
                    ps_os = {}
                    def warm_keeper():
                        for i in range(N_WARM):
                            nc.tensor.matmul(
                                ps_os[(0, 0)][:], zz[:],
                                qin[:, i % 8, 0:512],
                                start=False, stop=False)
                    for j in range(HPC):
                        if j == 1:
                            warm_keeper()
                        ev = a2a_out[j].rearrange("(a two) p q -> a two p q",
                                                  two=2)
                        nc.sync.dma_start(
                            ctx_p[j][0:64, :, :],
                            ev[:, 0].rearrange("a p q -> p a q"))
                        nc.sync.dma_start(
                            ctx_p[j][64:128, :, :],
                            ev[:, 1].rearrange("a p q -> p a q"))
                        for qc in range(SQ // 128):
                            for eh in range(2):
                                es = slice(eh * 512, (eh + 1) * 512)
                                if j == 0:
                                    ps_o = d_ps.tile([128, 512], F32,
                                                     tag="pso",
                                                     name=f"po{qc}{eh}")
                                    ps_os[(qc, eh)] = ps_o
                                else:
                                    ps_o = ps_os[(qc, eh)]
                                for a in range(4):
                                    nc.tensor.matmul(
                                        ps_o[:],
                                        ctx_p[j][:, a,
                                                 qc * 128:(qc + 1) * 128],
                                        wo_sb2[:, 4 * j + a, es],
                                        start=(j == 0 and a == 0),
                                        stop=(j == 1 and a == 3))
                                if j == 0:
                                    nc.tensor.matmul(
                                        ps_o[:], ones_bf[:, 0:128],
                                        bo_sb[:, es], start=False, stop=False)
                                else:
                                    o_sb = d_out.tile([128, 512], F32,
                                                      tag="osb")
                                    if (qc + eh) % 2:
                                        nc.vector.tensor_copy(o_sb[:],
                                                              ps_o[:])
                                    else:
                                        nc.scalar.copy(o_sb[:], ps_o[:])
                                    nc.sync.dma_start(
                                        out[qc * 128:(qc + 1) * 128, es],
                                        o_sb[:])

    nc.compile()
    return nc


def prepare(query, key, value, mask, Wq, bq, Wk, bk, Wv, bv, Wo, bo):
    """Host-side sharding/preprocessing + program build. Returns the compiled
    Bass program and the per-core input maps."""
    query = np.asarray(query)
    key = np.asarray(key)
    value = np.asarray(value)
    mask = np.asarray(mask)
    Wq, bq = np.asarray(Wq), np.asarray(bq)
    Wk, bk = np.asarray(Wk), np.asarray(bk)
    Wv, bv = np.asarray(Wv), np.asarray(bv)
    Wo, bo = np.asarray(Wo), np.asarray(bo)

    idx = np.nonzero(mask.reshape(-1) != 0)[0]
    n = int(idx.size)
    n_pad = max(512, ((n + 127) // 128) * 128)
    KC = n_pad // 128
    kc_real = n // 128
    kc_mixed = 1 if n % 128 else 0

    key_c = np.zeros((n_pad, D), np.float32)
    val_c = np.zeros((n_pad, D), np.float32)
    key_c[:n] = key[0, idx, :]
    val_c[:n] = value[0, idx, :]

    kcT_np = _bf16(key_c.T)
    vcT_np = _bf16(val_c.T)
    qT_np = _bf16(query[0].T)
    wqT_np = _bf16(Wq.T)
    wkT_np = _bf16(Wk.T)
    wvT_np = _bf16(Wv.T)
    woT_r = Wo.T
    slots = []
    for j in range(HPC):
        for a in range(4):
            hA, hB = 4 * a + j, 4 * a + 2 + j
            slots.append(woT_r[64 * hA:64 * hA + 64, :])
            slots.append(woT_r[64 * hB:64 * hB + 64, :])
    woT_np = _bf16(np.concatenate(slots, axis=0))
    bo_r_np = _bf16(bo.reshape(1, D))

    pb_np = np.zeros((128, KC), np.float32)
    flat = np.full(n_pad, NEG_BIG, np.float32)
    flat[:n] = 0.0
    pb_np[:] = flat.reshape(KC, 128).T

    nc = build_program(n_pad, kc_real, kc_mixed)

    in_maps = []
    for m in range(N_CORES):
        sl = slice(m * 128, (m + 1) * 128)
        in_maps.append({
            "qT": qT_np,
            "kcT": kcT_np,
            "vcT": vcT_np,
            "wqT": np.ascontiguousarray(wqT_np[:, sl]),
            "wkT": np.ascontiguousarray(wkT_np[:, sl]),
            "wvT": np.ascontiguousarray(wvT_np[:, sl]),
            "woT": woT_np,
            "bq_m": np.ascontiguousarray(
                bq[sl].reshape(128, 1).astype(np.float32)),
            "bk_m": np.ascontiguousarray(
                bk[sl].reshape(128, 1).astype(np.float32)),
            "bv_m": _bf16(bv[sl].reshape(1, 128)),
            "bo_r": bo_r_np,
            "pbias": pb_np,
        })

    return {"nc": nc, "in_maps": in_maps, "n": n, "n_pad": n_pad}


def kernel(query, key, value, mask, Wq, bq, Wk, bk, Wv, bv, Wo, bo,
           _trace=False, _result_box=None):
    prep = prepare(query, key, value, mask, Wq, bq, Wk, bk, Wv, bv, Wo, bo)
    res = run_bass_kernel_spmd(prep["nc"], prep["in_maps"],
                               list(range(N_CORES)), trace=_trace)
    if _result_box is not None:
        _result_box.append(res)

    out = np.concatenate([res.results[m]["out"] for m in range(N_CORES)],
                         axis=0)
    return out.reshape(1, S, D).astype(np.float32)



# revision 7
# speedup vs baseline: 1.0460x; 1.0154x over previous
"""Multi-head attention (B=1, S=4096, D=1024, H=16) on 8 TRN2 NeuronCores.

Strategy (head-sharded attention + AllToAll context exchange):
  - Host: compact K/V to the unmasked key positions (mask==0 keys contribute
    exactly 0 to softmax numerator and denominator, since the reference's
    -1e9 masking underflows exp to 0.0), transpose activations/weights to
    feature-major, cast matmul operands to bf16.
  - Phase A: core m computes K^T and V projections for its 2 heads over all
    compacted positions; results stay in SBUF (no gather needed).
  - Phase B: Q projection for the same 2 heads over ALL 4096 queries.
  - Phase C: attention for the 2 heads x 4096 queries: scores^T =
    K^T-chunk.T @ Q^T in PSUM ([k,q] layout, per-head via matmul
    tile_position row groups), exp on ScalarE straight out of PSUM (padding
    bias folded into the per-partition activation bias), P@V with a
    ones-augmented V (row 64 = softmax denominators), reciprocal + K=1
    broadcast matmul + multiply to normalize. Per-head context goes to DRAM
    sliced by query block.
  - AllToAll (one per head, 2 MiB, the first overlaps the second head's
    compute) converts head-sharding to query-sharding: afterwards core m
    holds all 16 heads' context for its own 512 queries.
  - Phase D: output projection of the core's 512 rows. The host just
    concatenates the 8 row-slices.
"""

import numpy as np
import ml_dtypes

import concourse.bacc as bacc
import concourse.mybir as mybir
import concourse.tile as tile
from concourse.bass_utils import run_bass_kernel_spmd

HEADS = 16
D = 1024
DH = 64
S = 4096
N_CORES = 8
SQ = S // N_CORES          # query rows owned per core (output sharding)
HPC = HEADS // N_CORES     # heads per core
BF16 = mybir.dt.bfloat16
F32 = mybir.dt.float32

NEG_BIG = -3840.0          # exp(-3840) == 0.0 exactly in fp32
EXP_GROUP = 2              # k-chunks (PSUM banks) per exp activation op
I16 = mybir.dt.int16
S_BUFS = 3
A_BUFS = 3
EXPJ = 0
QIN_SPLITS = 2
N_WARM = 111
N_WARM_A = 38
WO_SPLITS = 1
WO_DELAY = 0
CTXP_SPLIT = 0
DEFER_FIN = 1
FIN_AT = 2
KV_INTERLEAVE = 1
CTX_BUFS = 2
CEXP_BUFS = 5
CMISC_BUFS = 3
LOG2E = 1.4426950408889634
A16 = 0.125 * 128.0 * LOG2E   # DVE exp trick: bf16 = i16(x*A16 + B16)
B16 = 127.0 * 128.0


def _bf16(x):
    return np.ascontiguousarray(x.astype(ml_dtypes.bfloat16))


def build_program(n_pad, kc_real, kc_mixed, repeat=1, ablate=()):
    """Build the 8-core SPMD program.

    n_pad: padded compacted key count (multiple of 512).
    kc_real: number of leading k-chunks (of 128) with no padding.
    kc_mixed: 1 if a chunk straddles n (it gets a per-partition bias column
    on its exp); chunks past kc_real + kc_mixed are fully padded and get a
    constant NEG_BIG bias.
    """
    KC = n_pad // 128
    kblocks = []
    _b0 = 0
    while _b0 < n_pad:
        kblocks.append((_b0, min(512, n_pad - _b0)))
        _b0 += 512
    QC = S // 512            # query column groups (whole sequence)
    nc = bacc.Bacc("TRN2", target_bir_lowering=False, debug=False,
                   num_devices=N_CORES)

    # ---- I/O ----  (all bf16 unless noted; feature-major activations)
    qT = nc.dram_tensor("qT", [D, S], BF16, kind="ExternalInput")
    kcT = nc.dram_tensor("kcT", [D, n_pad], BF16, kind="ExternalInput")
    vcT = nc.dram_tensor("vcT", [D, n_pad], BF16, kind="ExternalInput")
    wqT = nc.dram_tensor("wqT", [D, HPC * DH], BF16, kind="ExternalInput")
    wkT = nc.dram_tensor("wkT", [D, HPC * DH], BF16, kind="ExternalInput")
    wvT = nc.dram_tensor("wvT", [D, HPC * DH], BF16, kind="ExternalInput")
    woT = nc.dram_tensor("woT", [D, D], BF16, kind="ExternalInput")
    bq_m = nc.dram_tensor("bq_m", [128, 1], F32, kind="ExternalInput")
    bk_m = nc.dram_tensor("bk_m", [128, 1], F32, kind="ExternalInput")
    bv_m = nc.dram_tensor("bv_m", [1, HPC * DH], BF16, kind="ExternalInput")
    bo_r = nc.dram_tensor("bo_r", [1, D], BF16, kind="ExternalInput")
    # per k-chunk exp bias column (0 for real keys, NEG_BIG for padding)
    pbias = nc.dram_tensor("pbias", [128, KC], F32, kind="ExternalInput")
    out = nc.dram_tensor("out", [SQ, D], F32, kind="ExternalOutput")

    with tile.TileContext(nc) as tc:
        for _rep in range(repeat):
            with (
                tc.tile_pool(name="dram", bufs=1, space="DRAM") as dram,
                tc.tile_pool(name="consts", bufs=1) as consts,
                tc.tile_pool(name="persist", bufs=1) as persist,
            ):
                # per-head A2A buffers: shard q-block -> [64 feats, 512 q]
                a2a_in = [dram.tile([N_CORES, 64, 512], BF16, name=f"a2i{j}")
                          for j in range(HPC)]
                a2a_out = [dram.tile([N_CORES, 64, 512], BF16, name=f"a2o{j}")
                           for j in range(HPC)]

                ones_bf = consts.tile([1, 128], BF16)
                nc.vector.memset(ones_bf[:], 1.0)
                ones_f = consts.tile([1, 64], F32)
                nc.vector.memset(ones_f[:], 1.0)
                bq_sb = consts.tile([128, 1], F32)
                nc.sync.dma_start(bq_sb[:], bq_m[:])
                bk_sb = consts.tile([128, 1], F32)
                nc.sync.dma_start(bk_sb[:], bk_m[:])
                bv_sb = consts.tile([1, HPC * DH], BF16)
                nc.sync.dma_start(bv_sb[:], bv_m[:])
                bo_sb = consts.tile([1, D], BF16)
                nc.sync.dma_start(bo_sb[:], bo_r[:])
                pb_sb = consts.tile([128, KC], F32)
                nc.sync.dma_start(pb_sb[:], pbias[:])

                kT_all = persist.tile([128, n_pad], BF16)
                wq_sb = persist.tile([128, 8, HPC * DH], BF16)
                q0_in = persist.tile([128, 8, 512], BF16)
                # v layout: [n-part, k-chunk, head, DH+1]; col DH == ones
                v_all = persist.tile([128, KC, HPC, DH + 1], BF16)
                q_pair = persist.tile([128, QC, 512], BF16)

                # ---------- Phase A: K/V projection (own 2 heads) ----------
                with (
                    tc.tile_pool(name="a_w", bufs=1) as a_w,
                    tc.tile_pool(name="a_in", bufs=1) as a_in,
                    tc.tile_pool(name="a_ps", bufs=A_BUFS, space="PSUM") as a_ps,
                    tc.tile_pool(name="a_psw", bufs=1, space="PSUM") as a_psw,
                ):
                    wk_sb = a_w.tile([128, 8, HPC * DH], BF16)
                    wv_sb = a_w.tile([128, 8, HPC * DH], BF16)
                    nc.sync.dma_start(wk_sb[:],
                                      wkT.rearrange("(c p) m -> p c m",
                                                    p=128))
                    nc.scalar.dma_start(wv_sb[:],
                                        wvT.rearrange("(c p) m -> p c m",
                                                      p=128))
                    nc.scalar.dma_start(wq_sb[:],
                                        wqT.rearrange("(c p) m -> p c m",
                                                      p=128))
                    nc.sync.dma_start(q0_in[:],
                                      qT[:, 0:512].rearrange(
                                          "(c p) m -> p c m", p=128))
                    nc.vector.memset(v_all[:, :, :, DH:DH + 1], 1.0)

                    # one fat contiguous DMA per 128-row chunk
                    kin = a_in.tile([128, 8, n_pad], BF16)
                    vin = a_in.tile([128, 8, n_pad], BF16)
                    if KV_INTERLEAVE:
                        nh = (n_pad // 2 + 511) // 512 * 512
                        nh = min(nh, n_pad)
                        nc.sync.dma_start(
                            kin[:, :, 0:nh],
                            kcT[:, 0:nh].rearrange("(c p) n -> p c n", p=128))
                        nc.scalar.dma_start(
                            vin[:, :, 0:nh],
                            vcT[:, 0:nh].rearrange("(c p) n -> p c n", p=128))
                        if nh < n_pad:
                            nc.sync.dma_start(
                                kin[:, :, nh:],
                                kcT[:, nh:].rearrange("(c p) n -> p c n",
                                                      p=128))
                            nc.scalar.dma_start(
                                vin[:, :, nh:],
                                vcT[:, nh:].rearrange("(c p) n -> p c n",
                                                      p=128))
                    else:
                        nc.sync.dma_start(kin[:],
                                          kcT.rearrange("(c p) n -> p c n",
                                                        p=128))
                        nc.scalar.dma_start(vin[:],
                                            vcT.rearrange("(c p) n -> p c n",
                                                          p=128))

                    if N_WARM_A:
                        wmov = a_w.tile([128, 512], BF16)
                        nc.vector.memset(wmov[:], 0.0)
                        wst = a_w.tile([128, 128], BF16)
                        nc.vector.memset(wst[:], 0.0)
                        ps_w = a_psw.tile([128, 512], F32, tag="pw")
                        for i in range(N_WARM_A):
                            nc.tensor.matmul(ps_w[:], wst[:], wmov[:],
                                             start=(i == 0),
                                             stop=(i == N_WARM_A - 1))
                    ps_q0 = a_ps.tile([128, 512], F32, tag="psk")
                    for c in range(8):
                        nc.tensor.matmul(ps_q0[:], wq_sb[:, c, :],
                                         q0_in[:, c, :],
                                         start=(c == 0), stop=(c == 7))
                    nc.vector.tensor_scalar_add(q_pair[:, 0, :], ps_q0[:],
                                                bq_sb[:])

                    for (b0, bw) in kblocks:
                        ns = slice(b0, b0 + bw)
                        ps_k = a_ps.tile([128, 512], F32, tag="psk")
                        for c in range(8):
                            nc.tensor.matmul(ps_k[:, 0:bw], wk_sb[:, c, :],
                                             kin[:, c, ns],
                                             start=(c == 0), stop=(c == 7))
                        nc.vector.tensor_scalar_add(kT_all[:, ns],
                                                    ps_k[:, 0:bw], bk_sb[:])
                    for kc in range(KC):
                        ks = slice(kc * 128, (kc + 1) * 128)
                        ps_v = a_ps.tile([128, HPC * DH], F32, tag="psv")
                        for c in range(8):
                            nc.tensor.matmul(
                                ps_v[:], vin[:, c, ks],
                                wv_sb[:, c, :], start=(c == 0), stop=False)
                        ps_v_done = nc.tensor.matmul(
                            ps_v[:], ones_bf[:, :128],
                            bv_sb[:], start=False, stop=True)
                        eng = nc.vector if kc % 2 else nc.scalar
                        if kc % 2:
                            nc.vector.tensor_copy(
                                v_all[:, kc, :, 0:DH],
                                ps_v[:].rearrange("p (j d) -> p j d", j=HPC))
                        else:
                            nc.scalar.copy(
                                v_all[:, kc, :, 0:DH],
                                ps_v[:].rearrange("p (j d) -> p j d", j=HPC))

                # ---------- Phase B folded into C: q blocks on demand --
                # behind kin/vin on the two HWDGE queues so the big qT load
                # cannot delay phase A's inputs
                qin = persist.tile([128, 8, S - 512], BF16)
                for qh in range(QIN_SPLITS):
                    w = (S - 512) // QIN_SPLITS
                    a, b = qh * w, (qh + 1) * w
                    nc.scalar.dma_start(
                        qin[:, :, a:b],
                        qT[:, 512 + a:512 + b].rearrange(
                            "(c p) m -> p c m", p=128))

                # ---------- Phase C: attention for own 2 heads ----------
                # wo is loaded early so phase D's weights are resident
                wo_sb2 = persist.tile([128, N_CORES, D], BF16)
                if WO_DELAY:
                    # hold the gpsimd queue until kin lands so the wo load
                    # cannot occupy the DMA device ahead of phase-A inputs
                    wo_gate = persist.tile([1, 8], BF16)
                    nc.gpsimd.tensor_copy(wo_gate[:], qin[0:1, :, 0:1])
                for wh in range(WO_SPLITS):
                    w = D // WO_SPLITS
                    a, b = wh * w, (wh + 1) * w
                    nc.gpsimd.dma_start(
                        wo_sb2[:, :, a:b],
                        woT[:, a:b].rearrange("(c p) m -> p c m", p=128))
                with (
                    tc.tile_pool(name="c_exp", bufs=CEXP_BUFS) as c_exp,
                    tc.tile_pool(name="c_misc", bufs=CMISC_BUFS) as c_misc,
                    tc.tile_pool(name="c_ps_s", bufs=S_BUFS, space="PSUM") as c_ps_s,
                    tc.tile_pool(name="c_ps_c", bufs=CTX_BUFS, space="PSUM") as c_ps_c,
                ):
                    pend_fin = []
                    for j in range(HPC):
                        pj = slice(64 * j, 64 * (j + 1))
                        for qc in range(QC):
                            if j == 0 and qc + 1 < QC:
                                qs = slice(qc * 512, (qc + 1) * 512)
                                ps_q = c_ps_s.tile([128, EXP_GROUP, 512], F32,
                                                   tag="s", name=f"psq{qc}")
                                for c in range(8):
                                    nc.tensor.matmul(
                                        ps_q[:, 0, :], wq_sb[:, c, :],
                                        qin[:, c, qs],
                                        start=(c == 0), stop=(c == 7))
                                nc.vector.tensor_scalar_add(
                                    q_pair[:, qc + 1, :], ps_q[:, 0, :],
                                    bq_sb[:])
                            ps_ctx = c_ps_c.tile([128, 512], F32, tag="ctx")
                            rhs_q = q_pair[pj, qc, :]
                            c0 = 0
                            gi = 0
                            while c0 < KC:
                                if gi == FIN_AT and pend_fin:
                                    pend_fin.pop(0)()
                                gn = min(EXP_GROUP, KC - c0)
                                ps_s = c_ps_s.tile([128, EXP_GROUP, 512], F32,
                                                   tag="s")
                                for cc in range(gn):
                                    lc = c0 + cc
                                    nc.tensor.matmul(
                                        ps_s[:, cc, :],
                                        kT_all[pj,
                                               lc * 128:(lc + 1) * 128],
                                        rhs_q, start=True, stop=True,
                                        tile_position=(64 * j, 0))
                                exp_sb = c_exp.tile([128, EXP_GROUP, 512],
                                                    BF16, tag="e")
                                clean = c0 + gn <= kc_real
                                if clean and (gi + qc + EXPJ * j) % 2 == 0:
                                    # exp2 bit-trick: bf16 == i16(x*A16+B16)
                                    nc.vector.tensor_scalar(
                                        exp_sb[:, 0:gn, :].bitcast(I16),
                                        ps_s[:, 0:gn, :], A16, B16,
                                        mybir.AluOpType.mult,
                                        mybir.AluOpType.add)
                                elif clean:
                                    nc.scalar.activation(
                                        exp_sb[:, 0:gn, :], ps_s[:, 0:gn, :],
                                        mybir.ActivationFunctionType.Exp,
                                        bias=0.0, scale=0.125)
                                else:
                                    for cc in range(gn):
                                        nc.scalar.activation(
                                            exp_sb[:, cc, :], ps_s[:, cc, :],
                                            mybir.ActivationFunctionType.Exp,
                                            bias=pb_sb[:, c0 + cc:c0 + cc + 1],
                                            scale=0.125)
                                for cc in range(gn):
                                    lc = c0 + cc
                                    nc.tensor.matmul(
                                        ps_ctx[0:DH + 1, :],
                                        v_all[:, lc, j, :],
                                        exp_sb[:, cc, :],
                                        start=(lc == 0),
                                        stop=(lc == KC - 1))
                                c0 += gn
                                gi += 1

                            def _fin(j=j, qc=qc, ps_ctx=ps_ctx):
                                recip = c_misc.tile([1, 512], BF16,
                                                    tag="recip", name="rc")
                                with nc.allow_low_precision(reason="1/d"):
                                    nc.vector.reciprocal(
                                        recip[:], ps_ctx[DH:DH + 1, :])
                                nc.tensor.matmul(
                                    ps_ctx[64:128, :],
                                    ones_bf[0:1, 0:64], recip[:],
                                    start=True, stop=True,
                                    tile_position=(0, 64),
                                    skip_group_check=True)
                                rec_bc = c_misc.tile([64, 512], F32,
                                                     tag="rbc", name="rb")
                                nc.scalar.copy(rec_bc[:], ps_ctx[64:128, :])
                                ctx_sb = c_misc.tile([64, 512], BF16,
                                                     tag="ctxs", name="cx")
                                nc.vector.tensor_mul(ctx_sb[:],
                                                     ps_ctx[0:64, :],
                                                     rec_bc[:])
                                nc.sync.dma_start(a2a_in[j][qc], ctx_sb[:])
                            if DEFER_FIN:
                                pend_fin.append(_fin)
                            else:
                                _fin()

                        while pend_fin:
                            pend_fin.pop(0)()
                        if "cclocal" in ablate:
                            nc.sync.dma_start(a2a_out[j][:], a2a_in[j][:])
                        else:
                            nc.gpsimd.collective_compute(
                                "AllToAll", mybir.AluOpType.bypass,
                                replica_groups=[list(range(N_CORES))],
                                ins=[a2a_in[j].opt()],
                                outs=[a2a_out[j].opt()])

                # ---------- Phase D: output projection (own 512 rows) ----------
                if "noD" in ablate:
                    continue
                with (
                    tc.tile_pool(name="d_w", bufs=1) as d_w,
                    tc.tile_pool(name="d_out", bufs=3) as d_out,
                    tc.tile_pool(name="d_ps", bufs=8, space="PSUM") as d_ps,
                ):
                    # heads of equal j stacked in pairs on partitions (K=128);
                    # D0 (j=0 pairs) depends only on A2A_0 and overlaps A2A_1.
                    # The 8 (qc, eh) PSUM tiles stay open across both j passes.
                    ctx_p = [d_w.tile([128, 4, 512], BF16, name=f"cxp{j}")
                             for j in range(HPC)]
                    zz = d_w.tile([128, 128], BF16)
                    nc.vector.memset(zz[:], 0.0)
                    ps_os = {}
                    def warm_keeper():
                        for i in range(N_WARM):
                            nc.tensor.matmul(
                                ps_os[(0, 0)][:], zz[:],
                                qin[:, i % 8, 0:512],
                                start=False, stop=False)
                    for j in range(HPC):
                        if j == 1:
                            warm_keeper()
                        ev = a2a_out[j].rearrange("(a two) p q -> a two p q",
                                                  two=2)
                        if j == 0 or not CTXP_SPLIT:
                            nc.sync.dma_start(
                                ctx_p[j][0:64, :, :],
                                ev[:, 0].rearrange("a p q -> p a q"))
                            nc.sync.dma_start(
                                ctx_p[j][64:128, :, :],
                                ev[:, 1].rearrange("a p q -> p a q"))
                        else:
                            for qq in range(4):
                                qs = slice(qq * 128, (qq + 1) * 128)
                                nc.sync.dma_start(
                                    ctx_p[j][0:64, :, qs],
                                    ev[:, 0, :, qs].rearrange(
                                        "a p q -> p a q"))
                                nc.scalar.dma_start(
                                    ctx_p[j][64:128, :, qs],
                                    ev[:, 1, :, qs].rearrange(
                                        "a p q -> p a q"))
                        for qc in range(SQ // 128):
                            for eh in range(2):
                                es = slice(eh * 512, (eh + 1) * 512)
                                if j == 0:
                                    ps_o = d_ps.tile([128, 512], F32,
                                                     tag="pso",
                                                     name=f"po{qc}{eh}")
                                    ps_os[(qc, eh)] = ps_o
                                else:
                                    ps_o = ps_os[(qc, eh)]
                                for a in range(4):
                                    nc.tensor.matmul(
                                        ps_o[:],
                                        ctx_p[j][:, a,
                                                 qc * 128:(qc + 1) * 128],
                                        wo_sb2[:, 4 * j + a, es],
                                        start=(j == 0 and a == 0),
                                        stop=(j == 1 and a == 3))
                                if j == 0:
                                    nc.tensor.matmul(
                                        ps_o[:], ones_bf[:, 0:128],
                                        bo_sb[:, es], start=False, stop=False)
                                else:
                                    o_sb = d_out.tile([128, 512], F32,
                                                      tag="osb")
                                    if (qc + eh) % 2:
                                        nc.vector.tensor_copy(o_sb[:],
                                                              ps_o[:])
                                    else:
                                        nc.scalar.copy(o_sb[:], ps_o[:])
                                    nc.sync.dma_start(
                                        out[qc * 128:(qc + 1) * 128, es],
                                        o_sb[:])

    nc.compile()
    return nc


def prepare(query, key, value, mask, Wq, bq, Wk, bk, Wv, bv, Wo, bo):
    """Host-side sharding/preprocessing + program build. Returns the compiled
    Bass program and the per-core input maps."""
    query = np.asarray(query)
    key = np.asarray(key)
    value = np.asarray(value)
    mask = np.asarray(mask)
    Wq, bq = np.asarray(Wq), np.asarray(bq)
    Wk, bk = np.asarray(Wk), np.asarray(bk)
    Wv, bv = np.asarray(Wv), np.asarray(bv)
    Wo, bo = np.asarray(Wo), np.asarray(bo)

    idx = np.nonzero(mask.reshape(-1) != 0)[0]
    n = int(idx.size)
    n_pad = max(512, ((n + 127) // 128) * 128)
    KC = n_pad // 128
    kc_real = n // 128
    kc_mixed = 1 if n % 128 else 0

    key_c = np.zeros((n_pad, D), np.float32)
    val_c = np.zeros((n_pad, D), np.float32)
    key_c[:n] = key[0, idx, :]
    val_c[:n] = value[0, idx, :]

    kcT_np = _bf16(key_c.T)
    vcT_np = _bf16(val_c.T)
    qT_np = _bf16(query[0].T)
    wqT_np = _bf16(Wq.T)
    wkT_np = _bf16(Wk.T)
    wvT_np = _bf16(Wv.T)
    woT_r = Wo.T
    slots = []
    for j in range(HPC):
        for a in range(4):
            hA, hB = 4 * a + j, 4 * a + 2 + j
            slots.append(woT_r[64 * hA:64 * hA + 64, :])
            slots.append(woT_r[64 * hB:64 * hB + 64, :])
    woT_np = _bf16(np.concatenate(slots, axis=0))
    bo_r_np = _bf16(bo.reshape(1, D))

    pb_np = np.zeros((128, KC), np.float32)
    flat = np.full(n_pad, NEG_BIG, np.float32)
    flat[:n] = 0.0
    pb_np[:] = flat.reshape(KC, 128).T

    nc = build_program(n_pad, kc_real, kc_mixed)

    in_maps = []
    for m in range(N_CORES):
        sl = slice(m * 128, (m + 1) * 128)
        in_maps.append({
            "qT": qT_np,
            "kcT": kcT_np,
            "vcT": vcT_np,
            "wqT": np.ascontiguousarray(wqT_np[:, sl]),
            "wkT": np.ascontiguousarray(wkT_np[:, sl]),
            "wvT": np.ascontiguousarray(wvT_np[:, sl]),
            "woT": woT_np,
            "bq_m": np.ascontiguousarray(
                bq[sl].reshape(128, 1).astype(np.float32)),
            "bk_m": np.ascontiguousarray(
                bk[sl].reshape(128, 1).astype(np.float32)),
            "bv_m": _bf16(bv[sl].reshape(1, 128)),
            "bo_r": bo_r_np,
            "pbias": pb_np,
        })

    return {"nc": nc, "in_maps": in_maps, "n": n, "n_pad": n_pad}


def kernel(query, key, value, mask, Wq, bq, Wk, bk, Wv, bv, Wo, bo,
           _trace=False, _result_box=None):
    prep = prepare(query, key, value, mask, Wq, bq, Wk, bk, Wv, bv, Wo, bo)
    res = run_bass_kernel_spmd(prep["nc"], prep["in_maps"],
                               list(range(N_CORES)), trace=_trace)
    if _result_box is not None:
        _result_box.append(res)

    out = np.concatenate([res.results[m]["out"] for m in range(N_CORES)],
                         axis=0)
    return out.reshape(1, S, D).astype(np.float32)



# revision 8
# speedup vs baseline: 1.0484x; 1.0023x over previous
"""Multi-head attention (B=1, S=4096, D=1024, H=16) on 8 TRN2 NeuronCores.

Strategy (head-sharded attention + AllToAll context exchange):
  - Host: compact K/V to the unmasked key positions (mask==0 keys contribute
    exactly 0 to softmax numerator and denominator, since the reference's
    -1e9 masking underflows exp to 0.0), transpose activations/weights to
    feature-major, cast matmul operands to bf16.
  - Phase A: core m computes K^T and V projections for its 2 heads over all
    compacted positions; results stay in SBUF (no gather needed).
  - Phase B: Q projection for the same 2 heads over ALL 4096 queries.
  - Phase C: attention for the 2 heads x 4096 queries: scores^T =
    K^T-chunk.T @ Q^T in PSUM ([k,q] layout, per-head via matmul
    tile_position row groups), exp on ScalarE straight out of PSUM (padding
    bias folded into the per-partition activation bias), P@V with a
    ones-augmented V (row 64 = softmax denominators), reciprocal + K=1
    broadcast matmul + multiply to normalize. Per-head context goes to DRAM
    sliced by query block.
  - AllToAll (one per head, 2 MiB, the first overlaps the second head's
    compute) converts head-sharding to query-sharding: afterwards core m
    holds all 16 heads' context for its own 512 queries.
  - Phase D: output projection of the core's 512 rows. The host just
    concatenates the 8 row-slices.
"""

import numpy as np
import ml_dtypes

import concourse.bacc as bacc
import concourse.mybir as mybir
import concourse.tile as tile
from concourse.bass_utils import run_bass_kernel_spmd

HEADS = 16
D = 1024
DH = 64
S = 4096
N_CORES = 8
SQ = S // N_CORES          # query rows owned per core (output sharding)
HPC = HEADS // N_CORES     # heads per core
BF16 = mybir.dt.bfloat16
F32 = mybir.dt.float32

NEG_BIG = -3840.0          # exp(-3840) == 0.0 exactly in fp32
EXP_GROUP = 2              # k-chunks (PSUM banks) per exp activation op
I16 = mybir.dt.int16
S_BUFS = 3
A_BUFS = 3
EXPJ = 0
QIN_SPLITS = 2
N_WARM = 111
N_WARM_A = 38
WO_SPLITS = 1
WO_DELAY = 0
CTXP_SPLIT = 0
DEFER_FIN = 1
FIN_AT = 2
QPROJ_AT = 0
KV_INTERLEAVE = 1
CTX_BUFS = 2
CEXP_BUFS = 9
CMISC_BUFS = 3
LOG2E = 1.4426950408889634
A16 = 0.125 * 128.0 * LOG2E   # DVE exp trick: bf16 = i16(x*A16 + B16)
B16 = 127.0 * 128.0


def _bf16(x):
    return np.ascontiguousarray(x.astype(ml_dtypes.bfloat16))


def build_program(n_pad, kc_real, kc_mixed, repeat=1, ablate=()):
    """Build the 8-core SPMD program.

    n_pad: padded compacted key count (multiple of 512).
    kc_real: number of leading k-chunks (of 128) with no padding.
    kc_mixed: 1 if a chunk straddles n (it gets a per-partition bias column
    on its exp); chunks past kc_real + kc_mixed are fully padded and get a
    constant NEG_BIG bias.
    """
    KC = n_pad // 128
    kblocks = []
    _b0 = 0
    while _b0 < n_pad:
        kblocks.append((_b0, min(512, n_pad - _b0)))
        _b0 += 512
    QC = S // 512            # query column groups (whole sequence)
    nc = bacc.Bacc("TRN2", target_bir_lowering=False, debug=False,
                   num_devices=N_CORES)

    # ---- I/O ----  (all bf16 unless noted; feature-major activations)
    qT = nc.dram_tensor("qT", [D, S], BF16, kind="ExternalInput")
    kcT = nc.dram_tensor("kcT", [D, n_pad], BF16, kind="ExternalInput")
    vcT = nc.dram_tensor("vcT", [D, n_pad], BF16, kind="ExternalInput")
    wqT = nc.dram_tensor("wqT", [D, HPC * DH], BF16, kind="ExternalInput")
    wkT = nc.dram_tensor("wkT", [D, HPC * DH], BF16, kind="ExternalInput")
    wvT = nc.dram_tensor("wvT", [D, HPC * DH], BF16, kind="ExternalInput")
    woT = nc.dram_tensor("woT", [D, D], BF16, kind="ExternalInput")
    bq_m = nc.dram_tensor("bq_m", [128, 1], F32, kind="ExternalInput")
    bk_m = nc.dram_tensor("bk_m", [128, 1], F32, kind="ExternalInput")
    bv_m = nc.dram_tensor("bv_m", [1, HPC * DH], BF16, kind="ExternalInput")
    bo_r = nc.dram_tensor("bo_r", [1, D], BF16, kind="ExternalInput")
    # per k-chunk exp bias column (0 for real keys, NEG_BIG for padding)
    pbias = nc.dram_tensor("pbias", [128, KC], F32, kind="ExternalInput")
    out = nc.dram_tensor("out", [SQ, D], F32, kind="ExternalOutput")

    with tile.TileContext(nc) as tc:
        for _rep in range(repeat):
            with (
                tc.tile_pool(name="dram", bufs=1, space="DRAM") as dram,
                tc.tile_pool(name="consts", bufs=1) as consts,
                tc.tile_pool(name="persist", bufs=1) as persist,
            ):
                # per-head A2A buffers: shard q-block -> [64 feats, 512 q]
                a2a_in = [dram.tile([N_CORES, 64, 512], BF16, name=f"a2i{j}")
                          for j in range(HPC)]
                a2a_out = [dram.tile([N_CORES, 64, 512], BF16, name=f"a2o{j}")
                           for j in range(HPC)]

                ones_bf = consts.tile([1, 128], BF16)
                nc.vector.memset(ones_bf[:], 1.0)
                ones_f = consts.tile([1, 64], F32)
                nc.vector.memset(ones_f[:], 1.0)
                bq_sb = consts.tile([128, 1], F32)
                nc.sync.dma_start(bq_sb[:], bq_m[:])
                bk_sb = consts.tile([128, 1], F32)
                nc.sync.dma_start(bk_sb[:], bk_m[:])
                bv_sb = consts.tile([1, HPC * DH], BF16)
                nc.sync.dma_start(bv_sb[:], bv_m[:])
                bo_sb = consts.tile([1, D], BF16)
                nc.sync.dma_start(bo_sb[:], bo_r[:])
                pb_sb = consts.tile([128, KC], F32)
                nc.sync.dma_start(pb_sb[:], pbias[:])

                kT_all = persist.tile([128, n_pad], BF16)
                wq_sb = persist.tile([128, 8, HPC * DH], BF16)
                q0_in = persist.tile([128, 8, 512], BF16)
                # v layout: [n-part, k-chunk, head, DH+1]; col DH == ones
                v_all = persist.tile([128, KC, HPC, DH + 1], BF16)
                q_pair = persist.tile([128, QC, 512], BF16)

                # ---------- Phase A: K/V projection (own 2 heads) ----------
                with (
                    tc.tile_pool(name="a_w", bufs=1) as a_w,
                    tc.tile_pool(name="a_in", bufs=1) as a_in,
                    tc.tile_pool(name="a_ps", bufs=A_BUFS, space="PSUM") as a_ps,
                    tc.tile_pool(name="a_psw", bufs=1, space="PSUM") as a_psw,
                ):
                    wk_sb = a_w.tile([128, 8, HPC * DH], BF16)
                    wv_sb = a_w.tile([128, 8, HPC * DH], BF16)
                    nc.sync.dma_start(wk_sb[:],
                                      wkT.rearrange("(c p) m -> p c m",
                                                    p=128))
                    nc.scalar.dma_start(wv_sb[:],
                                        wvT.rearrange("(c p) m -> p c m",
                                                      p=128))
                    nc.scalar.dma_start(wq_sb[:],
                                        wqT.rearrange("(c p) m -> p c m",
                                                      p=128))
                    nc.sync.dma_start(q0_in[:],
                                      qT[:, 0:512].rearrange(
                                          "(c p) m -> p c m", p=128))
                    nc.vector.memset(v_all[:, :, :, DH:DH + 1], 1.0)

                    # one fat contiguous DMA per 128-row chunk
                    kin = a_in.tile([128, 8, n_pad], BF16)
                    vin = a_in.tile([128, 8, n_pad], BF16)
                    if KV_INTERLEAVE:
                        nh = (n_pad // 2 + 511) // 512 * 512
                        nh = min(nh, n_pad)
                        nc.sync.dma_start(
                            kin[:, :, 0:nh],
                            kcT[:, 0:nh].rearrange("(c p) n -> p c n", p=128))
                        nc.scalar.dma_start(
                            vin[:, :, 0:nh],
                            vcT[:, 0:nh].rearrange("(c p) n -> p c n", p=128))
                        if nh < n_pad:
                            nc.sync.dma_start(
                                kin[:, :, nh:],
                                kcT[:, nh:].rearrange("(c p) n -> p c n",
                                                      p=128))
                            nc.scalar.dma_start(
                                vin[:, :, nh:],
                                vcT[:, nh:].rearrange("(c p) n -> p c n",
                                                      p=128))
                    else:
                        nc.sync.dma_start(kin[:],
                                          kcT.rearrange("(c p) n -> p c n",
                                                        p=128))
                        nc.scalar.dma_start(vin[:],
                                            vcT.rearrange("(c p) n -> p c n",
                                                          p=128))

                    if N_WARM_A:
                        wmov = a_w.tile([128, 512], BF16)
                        nc.vector.memset(wmov[:], 0.0)
                        wst = a_w.tile([128, 128], BF16)
                        nc.vector.memset(wst[:], 0.0)
                        ps_w = a_psw.tile([128, 512], F32, tag="pw")
                        for i in range(N_WARM_A):
                            nc.tensor.matmul(ps_w[:], wst[:], wmov[:],
                                             start=(i == 0),
                                             stop=(i == N_WARM_A - 1))
                    ps_q0 = a_ps.tile([128, 512], F32, tag="psk")
                    for c in range(8):
                        nc.tensor.matmul(ps_q0[:], wq_sb[:, c, :],
                                         q0_in[:, c, :],
                                         start=(c == 0), stop=(c == 7))
                    nc.vector.tensor_scalar_add(q_pair[:, 0, :], ps_q0[:],
                                                bq_sb[:])

                    for (b0, bw) in kblocks:
                        ns = slice(b0, b0 + bw)
                        ps_k = a_ps.tile([128, 512], F32, tag="psk")
                        for c in range(8):
                            nc.tensor.matmul(ps_k[:, 0:bw], wk_sb[:, c, :],
                                             kin[:, c, ns],
                                             start=(c == 0), stop=(c == 7))
                        nc.vector.tensor_scalar_add(kT_all[:, ns],
                                                    ps_k[:, 0:bw], bk_sb[:])
                    for kc in range(KC):
                        ks = slice(kc * 128, (kc + 1) * 128)
                        ps_v = a_ps.tile([128, HPC * DH], F32, tag="psv")
                        for c in range(8):
                            nc.tensor.matmul(
                                ps_v[:], vin[:, c, ks],
                                wv_sb[:, c, :], start=(c == 0), stop=False)
                        ps_v_done = nc.tensor.matmul(
                            ps_v[:], ones_bf[:, :128],
                            bv_sb[:], start=False, stop=True)
                        eng = nc.vector if kc % 2 else nc.scalar
                        if kc % 2:
                            nc.vector.tensor_copy(
                                v_all[:, kc, :, 0:DH],
                                ps_v[:].rearrange("p (j d) -> p j d", j=HPC))
                        else:
                            nc.scalar.copy(
                                v_all[:, kc, :, 0:DH],
                                ps_v[:].rearrange("p (j d) -> p j d", j=HPC))

                # ---------- Phase B folded into C: q blocks on demand --
                # behind kin/vin on the two HWDGE queues so the big qT load
                # cannot delay phase A's inputs
                qin = persist.tile([128, 8, S - 512], BF16)
                for qh in range(QIN_SPLITS):
                    w = (S - 512) // QIN_SPLITS
                    a, b = qh * w, (qh + 1) * w
                    nc.scalar.dma_start(
                        qin[:, :, a:b],
                        qT[:, 512 + a:512 + b].rearrange(
                            "(c p) m -> p c m", p=128))

                # ---------- Phase C: attention for own 2 heads ----------
                # wo is loaded early so phase D's weights are resident
                wo_sb2 = persist.tile([128, N_CORES, D], BF16)
                if WO_DELAY:
                    # hold the gpsimd queue until kin lands so the wo load
                    # cannot occupy the DMA device ahead of phase-A inputs
                    wo_gate = persist.tile([1, 8], BF16)
                    nc.gpsimd.tensor_copy(wo_gate[:], qin[0:1, :, 0:1])
                for wh in range(WO_SPLITS):
                    w = D // WO_SPLITS
                    a, b = wh * w, (wh + 1) * w
                    nc.gpsimd.dma_start(
                        wo_sb2[:, :, a:b],
                        woT[:, a:b].rearrange("(c p) m -> p c m", p=128))
                with (
                    tc.tile_pool(name="c_exp", bufs=CEXP_BUFS) as c_exp,
                    tc.tile_pool(name="c_misc", bufs=CMISC_BUFS) as c_misc,
                    tc.tile_pool(name="c_ps_s", bufs=S_BUFS, space="PSUM") as c_ps_s,
                    tc.tile_pool(name="c_ps_c", bufs=CTX_BUFS, space="PSUM") as c_ps_c,
                ):
                    pend_fin = []
                    for j in range(HPC):
                        pj = slice(64 * j, 64 * (j + 1))
                        for qc in range(QC):
                            def _qproj(qc=qc):
                                qs = slice(qc * 512, (qc + 1) * 512)
                                ps_q = c_ps_s.tile([128, EXP_GROUP, 512], F32,
                                                   tag="s", name=f"psq{qc}")
                                for c in range(8):
                                    nc.tensor.matmul(
                                        ps_q[:, 0, :], wq_sb[:, c, :],
                                        qin[:, c, qs],
                                        start=(c == 0), stop=(c == 7))
                                nc.vector.tensor_scalar_add(
                                    q_pair[:, qc + 1, :], ps_q[:, 0, :],
                                    bq_sb[:])
                            qp = _qproj if (j == 0 and qc + 1 < QC) else None
                            if QPROJ_AT == 0 and qp:
                                qp()
                                qp = None
                            ps_ctx = c_ps_c.tile([128, 512], F32, tag="ctx")
                            rhs_q = q_pair[pj, qc, :]
                            c0 = 0
                            gi = 0
                            while c0 < KC:
                                if gi == FIN_AT and pend_fin:
                                    pend_fin.pop(0)()
                                if gi == QPROJ_AT and qp:
                                    qp()
                                    qp = None
                                gn = min(EXP_GROUP, KC - c0)
                                ps_s = c_ps_s.tile([128, EXP_GROUP, 512], F32,
                                                   tag="s")
                                for cc in range(gn):
                                    lc = c0 + cc
                                    nc.tensor.matmul(
                                        ps_s[:, cc, :],
                                        kT_all[pj,
                                               lc * 128:(lc + 1) * 128],
                                        rhs_q, start=True, stop=True,
                                        tile_position=(64 * j, 0))
                                exp_sb = c_exp.tile([128, EXP_GROUP, 512],
                                                    BF16, tag="e")
                                clean = c0 + gn <= kc_real
                                if clean and (gi + qc + EXPJ * j) % 2 == 0:
                                    # exp2 bit-trick: bf16 == i16(x*A16+B16)
                                    nc.vector.tensor_scalar(
                                        exp_sb[:, 0:gn, :].bitcast(I16),
                                        ps_s[:, 0:gn, :], A16, B16,
                                        mybir.AluOpType.mult,
                                        mybir.AluOpType.add)
                                elif clean:
                                    nc.scalar.activation(
                                        exp_sb[:, 0:gn, :], ps_s[:, 0:gn, :],
                                        mybir.ActivationFunctionType.Exp,
                                        bias=0.0, scale=0.125)
                                else:
                                    for cc in range(gn):
                                        nc.scalar.activation(
                                            exp_sb[:, cc, :], ps_s[:, cc, :],
                                            mybir.ActivationFunctionType.Exp,
                                            bias=pb_sb[:, c0 + cc:c0 + cc + 1],
                                            scale=0.125)
                                for cc in range(gn):
                                    lc = c0 + cc
                                    nc.tensor.matmul(
                                        ps_ctx[0:DH + 1, :],
                                        v_all[:, lc, j, :],
                                        exp_sb[:, cc, :],
                                        start=(lc == 0),
                                        stop=(lc == KC - 1))
                                c0 += gn
                                gi += 1

                            def _fin(j=j, qc=qc, ps_ctx=ps_ctx):
                                recip = c_misc.tile([1, 512], BF16,
                                                    tag="recip", name="rc")
                                with nc.allow_low_precision(reason="1/d"):
                                    nc.vector.reciprocal(
                                        recip[:], ps_ctx[DH:DH + 1, :])
                                nc.tensor.matmul(
                                    ps_ctx[64:128, :],
                                    ones_bf[0:1, 0:64], recip[:],
                                    start=True, stop=True,
                                    tile_position=(0, 64),
                                    skip_group_check=True)
                                rec_bc = c_misc.tile([64, 512], F32,
                                                     tag="rbc", name="rb")
                                nc.scalar.copy(rec_bc[:], ps_ctx[64:128, :])
                                ctx_sb = c_misc.tile([64, 512], BF16,
                                                     tag="ctxs", name="cx")
                                nc.vector.tensor_mul(ctx_sb[:],
                                                     ps_ctx[0:64, :],
                                                     rec_bc[:])
                                nc.sync.dma_start(a2a_in[j][qc], ctx_sb[:])
                            if DEFER_FIN:
                                pend_fin.append(_fin)
                            else:
                                _fin()

                        while pend_fin:
                            pend_fin.pop(0)()
                        if "cclocal" in ablate:
                            nc.sync.dma_start(a2a_out[j][:], a2a_in[j][:])
                        else:
                            nc.gpsimd.collective_compute(
                                "AllToAll", mybir.AluOpType.bypass,
                                replica_groups=[list(range(N_CORES))],
                                ins=[a2a_in[j].opt()],
                                outs=[a2a_out[j].opt()])

                # ---------- Phase D: output projection (own 512 rows) ----------
                if "noD" in ablate:
                    continue
                with (
                    tc.tile_pool(name="d_w", bufs=1) as d_w,
                    tc.tile_pool(name="d_out", bufs=3) as d_out,
                    tc.tile_pool(name="d_ps", bufs=8, space="PSUM") as d_ps,
                ):
                    # heads of equal j stacked in pairs on partitions (K=128);
                    # D0 (j=0 pairs) depends only on A2A_0 and overlaps A2A_1.
                    # The 8 (qc, eh) PSUM tiles stay open across both j passes.
                    ctx_p = [d_w.tile([128, 4, 512], BF16, name=f"cxp{j}")
                             for j in range(HPC)]
                    zz = d_w.tile([128, 128], BF16)
                    nc.vector.memset(zz[:], 0.0)
                    ps_os = {}
                    def warm_keeper():
                        for i in range(N_WARM):
                            nc.tensor.matmul(
                                ps_os[(0, 0)][:], zz[:],
                                qin[:, i % 8, 0:512],
                                start=False, stop=False)
                    for j in range(HPC):
                        if j == 1:
                            warm_keeper()
                        ev = a2a_out[j].rearrange("(a two) p q -> a two p q",
                                                  two=2)
                        if j == 0 or not CTXP_SPLIT:
                            nc.sync.dma_start(
                                ctx_p[j][0:64, :, :],
                                ev[:, 0].rearrange("a p q -> p a q"))
                            nc.sync.dma_start(
                                ctx_p[j][64:128, :, :],
                                ev[:, 1].rearrange("a p q -> p a q"))
                        else:
                            for qq in range(4):
                                qs = slice(qq * 128, (qq + 1) * 128)
                                nc.sync.dma_start(
                                    ctx_p[j][0:64, :, qs],
                                    ev[:, 0, :, qs].rearrange(
                                        "a p q -> p a q"))
                                nc.scalar.dma_start(
                                    ctx_p[j][64:128, :, qs],
                                    ev[:, 1, :, qs].rearrange(
                                        "a p q -> p a q"))
                        for qc in range(SQ // 128):
                            for eh in range(2):
                                es = slice(eh * 512, (eh + 1) * 512)
                                if j == 0:
                                    ps_o = d_ps.tile([128, 512], F32,
                                                     tag="pso",
                                                     name=f"po{qc}{eh}")
                                    ps_os[(qc, eh)] = ps_o
                                else:
                                    ps_o = ps_os[(qc, eh)]
                                for a in range(4):
                                    nc.tensor.matmul(
                                        ps_o[:],
                                        ctx_p[j][:, a,
                                                 qc * 128:(qc + 1) * 128],
                                        wo_sb2[:, 4 * j + a, es],
                                        start=(j == 0 and a == 0),
                                        stop=(j == 1 and a == 3))
                                if j == 0:
                                    nc.tensor.matmul(
                                        ps_o[:], ones_bf[:, 0:128],
                                        bo_sb[:, es], start=False, stop=False)
                                else:
                                    o_sb = d_out.tile([128, 512], F32,
                                                      tag="osb")
                                    if (qc + eh) % 2:
                                        nc.vector.tensor_copy(o_sb[:],
                                                              ps_o[:])
                                    else:
                                        nc.scalar.copy(o_sb[:], ps_o[:])
                                    nc.sync.dma_start(
                                        out[qc * 128:(qc + 1) * 128, es],
                                        o_sb[:])

    nc.compile()
    return nc


def prepare(query, key, value, mask, Wq, bq, Wk, bk, Wv, bv, Wo, bo):
    """Host-side sharding/preprocessing + program build. Returns the compiled
    Bass program and the per-core input maps."""
    query = np.asarray(query)
    key = np.asarray(key)
    value = np.asarray(value)
    mask = np.asarray(mask)
    Wq, bq = np.asarray(Wq), np.asarray(bq)
    Wk, bk = np.asarray(Wk), np.asarray(bk)
    Wv, bv = np.asarray(Wv), np.asarray(bv)
    Wo, bo = np.asarray(Wo), np.asarray(bo)

    idx = np.nonzero(mask.reshape(-1) != 0)[0]
    n = int(idx.size)
    n_pad = max(512, ((n + 127) // 128) * 128)
    KC = n_pad // 128
    kc_real = n // 128
    kc_mixed = 1 if n % 128 else 0

    key_c = np.zeros((n_pad, D), np.float32)
    val_c = np.zeros((n_pad, D), np.float32)
    key_c[:n] = key[0, idx, :]
    val_c[:n] = value[0, idx, :]

    kcT_np = _bf16(key_c.T)
    vcT_np = _bf16(val_c.T)
    qT_np = _bf16(query[0].T)
    wqT_np = _bf16(Wq.T)
    wkT_np = _bf16(Wk.T)
    wvT_np = _bf16(Wv.T)
    woT_r = Wo.T
    slots = []
    for j in range(HPC):
        for a in range(4):
            hA, hB = 4 * a + j, 4 * a + 2 + j
            slots.append(woT_r[64 * hA:64 * hA + 64, :])
            slots.append(woT_r[64 * hB:64 * hB + 64, :])
    woT_np = _bf16(np.concatenate(slots, axis=0))
    bo_r_np = _bf16(bo.reshape(1, D))

    pb_np = np.zeros((128, KC), np.float32)
    flat = np.full(n_pad, NEG_BIG, np.float32)
    flat[:n] = 0.0
    pb_np[:] = flat.reshape(KC, 128).T

    nc = build_program(n_pad, kc_real, kc_mixed)

    in_maps = []
    for m in range(N_CORES):
        sl = slice(m * 128, (m + 1) * 128)
        in_maps.append({
            "qT": qT_np,
            "kcT": kcT_np,
            "vcT": vcT_np,
            "wqT": np.ascontiguousarray(wqT_np[:, sl]),
            "wkT": np.ascontiguousarray(wkT_np[:, sl]),
            "wvT": np.ascontiguousarray(wvT_np[:, sl]),
            "woT": woT_np,
            "bq_m": np.ascontiguousarray(
                bq[sl].reshape(128, 1).astype(np.float32)),
            "bk_m": np.ascontiguousarray(
                bk[sl].reshape(128, 1).astype(np.float32)),
            "bv_m": _bf16(bv[sl].reshape(1, 128)),
            "bo_r": bo_r_np,
            "pbias": pb_np,
        })

    return {"nc": nc, "in_maps": in_maps, "n": n, "n_pad": n_pad}


def kernel(query, key, value, mask, Wq, bq, Wk, bk, Wv, bv, Wo, bo,
           _trace=False, _result_box=None):
    prep = prepare(query, key, value, mask, Wq, bq, Wk, bk, Wv, bv, Wo, bo)
    res = run_bass_kernel_spmd(prep["nc"], prep["in_maps"],
                               list(range(N_CORES)), trace=_trace)
    if _result_box is not None:
        _result_box.append(res)

    out = np.concatenate([res.results[m]["out"] for m in range(N_CORES)],
                         axis=0)
    return out.reshape(1, S, D).astype(np.float32)



# revision 9
# speedup vs baseline: 1.0535x; 1.0048x over previous
"""Multi-head attention (B=1, S=4096, D=1024, H=16) on 8 TRN2 NeuronCores.

Strategy (head-sharded attention + AllToAll context exchange):
  - Host: compact K/V to the unmasked key positions (mask==0 keys contribute
    exactly 0 to softmax numerator and denominator, since the reference's
    -1e9 masking underflows exp to 0.0), transpose activations/weights to
    feature-major, cast matmul operands to bf16.
  - Phase A: core m computes K^T and V projections for its 2 heads over all
    compacted positions; results stay in SBUF (no gather needed).
  - Phase B: Q projection for the same 2 heads over ALL 4096 queries.
  - Phase C: attention for the 2 heads x 4096 queries: scores^T =
    K^T-chunk.T @ Q^T in PSUM ([k,q] layout, per-head via matmul
    tile_position row groups), exp on ScalarE straight out of PSUM (padding
    bias folded into the per-partition activation bias), P@V with a
    ones-augmented V (row 64 = softmax denominators), reciprocal + K=1
    broadcast matmul + multiply to normalize. Per-head context goes to DRAM
    sliced by query block.
  - AllToAll (one per head, 2 MiB, the first overlaps the second head's
    compute) converts head-sharding to query-sharding: afterwards core m
    holds all 16 heads' context for its own 512 queries.
  - Phase D: output projection of the core's 512 rows. The host just
    concatenates the 8 row-slices.
"""

import numpy as np
import ml_dtypes

import concourse.bacc as bacc
import concourse.mybir as mybir
import concourse.tile as tile
from concourse.bass_utils import run_bass_kernel_spmd

HEADS = 16
D = 1024
DH = 64
S = 4096
N_CORES = 8
SQ = S // N_CORES          # query rows owned per core (output sharding)
HPC = HEADS // N_CORES     # heads per core
BF16 = mybir.dt.bfloat16
F32 = mybir.dt.float32

NEG_BIG = -3840.0          # exp(-3840) == 0.0 exactly in fp32
EXP_GROUP = 2              # k-chunks (PSUM banks) per exp activation op
I16 = mybir.dt.int16
S_BUFS = 3
A_BUFS = 3
EXPJ = 0
QIN_SPLITS = 5
N_WARM = 111
N_WARM_A = 38
WO_SPLITS = 1
WO_DELAY = 0
CTXP_SPLIT = 0
DEFER_FIN = 1
FIN_AT = 2
EXP_PAT = lambda gi, qc, j: (gi + qc) % 2 == 0
QPROJ_AT = 0
KV_INTERLEAVE = 1
CTX_BUFS = 2
CEXP_BUFS = 9
CMISC_BUFS = 3
LOG2E = 1.4426950408889634
A16 = 0.125 * 128.0 * LOG2E   # DVE exp trick: bf16 = i16(x*A16 + B16)
B16 = 127.0 * 128.0


def _bf16(x):
    return np.ascontiguousarray(x.astype(ml_dtypes.bfloat16))


def build_program(n_pad, kc_real, kc_mixed, repeat=1, ablate=()):
    """Build the 8-core SPMD program.

    n_pad: padded compacted key count (multiple of 512).
    kc_real: number of leading k-chunks (of 128) with no padding.
    kc_mixed: 1 if a chunk straddles n (it gets a per-partition bias column
    on its exp); chunks past kc_real + kc_mixed are fully padded and get a
    constant NEG_BIG bias.
    """
    KC = n_pad // 128
    kblocks = []
    _b0 = 0
    while _b0 < n_pad:
        kblocks.append((_b0, min(512, n_pad - _b0)))
        _b0 += 512
    QC = S // 512            # query column groups (whole sequence)
    nc = bacc.Bacc("TRN2", target_bir_lowering=False, debug=False,
                   num_devices=N_CORES)

    # ---- I/O ----  (all bf16 unless noted; feature-major activations)
    qT = nc.dram_tensor("qT", [D, S], BF16, kind="ExternalInput")
    kcT = nc.dram_tensor("kcT", [D, n_pad], BF16, kind="ExternalInput")
    vcT = nc.dram_tensor("vcT", [D, n_pad], BF16, kind="ExternalInput")
    wqT = nc.dram_tensor("wqT", [D, HPC * DH], BF16, kind="ExternalInput")
    wkT = nc.dram_tensor("wkT", [D, HPC * DH], BF16, kind="ExternalInput")
    wvT = nc.dram_tensor("wvT", [D, HPC * DH], BF16, kind="ExternalInput")
    woT = nc.dram_tensor("woT", [D, D], BF16, kind="ExternalInput")
    bq_m = nc.dram_tensor("bq_m", [128, 1], F32, kind="ExternalInput")
    bk_m = nc.dram_tensor("bk_m", [128, 1], F32, kind="ExternalInput")
    bv_m = nc.dram_tensor("bv_m", [1, HPC * DH], BF16, kind="ExternalInput")
    bo_r = nc.dram_tensor("bo_r", [1, D], BF16, kind="ExternalInput")
    # per k-chunk exp bias column (0 for real keys, NEG_BIG for padding)
    pbias = nc.dram_tensor("pbias", [128, KC], F32, kind="ExternalInput")
    out = nc.dram_tensor("out", [SQ, D], F32, kind="ExternalOutput")

    with tile.TileContext(nc) as tc:
        for _rep in range(repeat):
            with (
                tc.tile_pool(name="dram", bufs=1, space="DRAM") as dram,
                tc.tile_pool(name="consts", bufs=1) as consts,
                tc.tile_pool(name="persist", bufs=1) as persist,
            ):
                # per-head A2A buffers: shard q-block -> [64 feats, 512 q]
                a2a_in = [dram.tile([N_CORES, 64, 512], BF16, name=f"a2i{j}")
                          for j in range(HPC)]
                a2a_out = [dram.tile([N_CORES, 64, 512], BF16, name=f"a2o{j}")
                           for j in range(HPC)]

                ones_bf = consts.tile([1, 128], BF16)
                nc.vector.memset(ones_bf[:], 1.0)
                ones_f = consts.tile([1, 64], F32)
                nc.vector.memset(ones_f[:], 1.0)
                bq_sb = consts.tile([128, 1], F32)
                nc.sync.dma_start(bq_sb[:], bq_m[:])
                bk_sb = consts.tile([128, 1], F32)
                nc.sync.dma_start(bk_sb[:], bk_m[:])
                bv_sb = consts.tile([1, HPC * DH], BF16)
                nc.sync.dma_start(bv_sb[:], bv_m[:])
                bo_sb = consts.tile([1, D], BF16)
                nc.sync.dma_start(bo_sb[:], bo_r[:])
                pb_sb = consts.tile([128, KC], F32)
                nc.sync.dma_start(pb_sb[:], pbias[:])

                kT_all = persist.tile([128, n_pad], BF16)
                wq_sb = persist.tile([128, 8, HPC * DH], BF16)
                q0_in = persist.tile([128, 8, 512], BF16)
                # v layout: [n-part, k-chunk, head, DH+1]; col DH == ones
                v_all = persist.tile([128, KC, HPC, DH + 1], BF16)
                q_pair = persist.tile([128, QC, 512], BF16)

                # ---------- Phase A: K/V projection (own 2 heads) ----------
                with (
                    tc.tile_pool(name="a_w", bufs=1) as a_w,
                    tc.tile_pool(name="a_in", bufs=1) as a_in,
                    tc.tile_pool(name="a_ps", bufs=A_BUFS, space="PSUM") as a_ps,
                    tc.tile_pool(name="a_psw", bufs=1, space="PSUM") as a_psw,
                ):
                    wk_sb = a_w.tile([128, 8, HPC * DH], BF16)
                    wv_sb = a_w.tile([128, 8, HPC * DH], BF16)
                    nc.sync.dma_start(wk_sb[:],
                                      wkT.rearrange("(c p) m -> p c m",
                                                    p=128))
                    nc.scalar.dma_start(wv_sb[:],
                                        wvT.rearrange("(c p) m -> p c m",
                                                      p=128))
                    nc.scalar.dma_start(wq_sb[:],
                                        wqT.rearrange("(c p) m -> p c m",
                                                      p=128))
                    nc.sync.dma_start(q0_in[:],
                                      qT[:, 0:512].rearrange(
                                          "(c p) m -> p c m", p=128))
                    nc.vector.memset(v_all[:, :, :, DH:DH + 1], 1.0)

                    # one fat contiguous DMA per 128-row chunk
                    kin = a_in.tile([128, 8, n_pad], BF16)
                    vin = a_in.tile([128, 8, n_pad], BF16)
                    if KV_INTERLEAVE:
                        nh = (n_pad // 2 + 511) // 512 * 512
                        nh = min(nh, n_pad)
                        nc.sync.dma_start(
                            kin[:, :, 0:nh],
                            kcT[:, 0:nh].rearrange("(c p) n -> p c n", p=128))
                        nc.scalar.dma_start(
                            vin[:, :, 0:nh],
                            vcT[:, 0:nh].rearrange("(c p) n -> p c n", p=128))
                        if nh < n_pad:
                            nc.sync.dma_start(
                                kin[:, :, nh:],
                                kcT[:, nh:].rearrange("(c p) n -> p c n",
                                                      p=128))
                            nc.scalar.dma_start(
                                vin[:, :, nh:],
                                vcT[:, nh:].rearrange("(c p) n -> p c n",
                                                      p=128))
                    else:
                        nc.sync.dma_start(kin[:],
                                          kcT.rearrange("(c p) n -> p c n",
                                                        p=128))
                        nc.scalar.dma_start(vin[:],
                                            vcT.rearrange("(c p) n -> p c n",
                                                          p=128))

                    if N_WARM_A:
                        wmov = a_w.tile([128, 512], BF16)
                        nc.vector.memset(wmov[:], 0.0)
                        wst = a_w.tile([128, 128], BF16)
                        nc.vector.memset(wst[:], 0.0)
                        ps_w = a_psw.tile([128, 512], F32, tag="pw")
                        for i in range(N_WARM_A):
                            nc.tensor.matmul(ps_w[:], wst[:], wmov[:],
                                             start=(i == 0),
                                             stop=(i == N_WARM_A - 1))
                    ps_q0 = a_ps.tile([128, 512], F32, tag="psk")
                    for c in range(8):
                        nc.tensor.matmul(ps_q0[:], wq_sb[:, c, :],
                                         q0_in[:, c, :],
                                         start=(c == 0), stop=(c == 7))
                    nc.vector.tensor_scalar_add(q_pair[:, 0, :], ps_q0[:],
                                                bq_sb[:])

                    for (b0, bw) in kblocks:
                        ns = slice(b0, b0 + bw)
                        ps_k = a_ps.tile([128, 512], F32, tag="psk")
                        for c in range(8):
                            nc.tensor.matmul(ps_k[:, 0:bw], wk_sb[:, c, :],
                                             kin[:, c, ns],
                                             start=(c == 0), stop=(c == 7))
                        nc.vector.tensor_scalar_add(kT_all[:, ns],
                                                    ps_k[:, 0:bw], bk_sb[:])
                    for kc in range(KC):
                        ks = slice(kc * 128, (kc + 1) * 128)
                        ps_v = a_ps.tile([128, HPC * DH], F32, tag="psv")
                        for c in range(8):
                            nc.tensor.matmul(
                                ps_v[:], vin[:, c, ks],
                                wv_sb[:, c, :], start=(c == 0), stop=False)
                        ps_v_done = nc.tensor.matmul(
                            ps_v[:], ones_bf[:, :128],
                            bv_sb[:], start=False, stop=True)
                        eng = nc.vector if kc % 2 else nc.scalar
                        if kc % 2:
                            nc.vector.tensor_copy(
                                v_all[:, kc, :, 0:DH],
                                ps_v[:].rearrange("p (j d) -> p j d", j=HPC))
                        else:
                            nc.scalar.copy(
                                v_all[:, kc, :, 0:DH],
                                ps_v[:].rearrange("p (j d) -> p j d", j=HPC))

                # ---------- Phase B folded into C: q blocks on demand --
                # behind kin/vin on the two HWDGE queues so the big qT load
                # cannot delay phase A's inputs
                qin = persist.tile([128, 8, S - 512], BF16)
                for qh in range(QIN_SPLITS):
                    w = (S - 512) // QIN_SPLITS
                    a = qh * w
                    b = (qh + 1) * w if qh < QIN_SPLITS - 1 else S - 512
                    nc.scalar.dma_start(
                        qin[:, :, a:b],
                        qT[:, 512 + a:512 + b].rearrange(
                            "(c p) m -> p c m", p=128))

                # ---------- Phase C: attention for own 2 heads ----------
                # wo is loaded early so phase D's weights are resident
                wo_sb2 = persist.tile([128, N_CORES, D], BF16)
                if WO_DELAY:
                    # hold the gpsimd queue until kin lands so the wo load
                    # cannot occupy the DMA device ahead of phase-A inputs
                    wo_gate = persist.tile([1, 8], BF16)
                    nc.gpsimd.tensor_copy(wo_gate[:], qin[0:1, :, 0:1])
                for wh in range(WO_SPLITS):
                    w = D // WO_SPLITS
                    a, b = wh * w, (wh + 1) * w
                    nc.gpsimd.dma_start(
                        wo_sb2[:, :, a:b],
                        woT[:, a:b].rearrange("(c p) m -> p c m", p=128))
                with (
                    tc.tile_pool(name="c_exp", bufs=CEXP_BUFS) as c_exp,
                    tc.tile_pool(name="c_misc", bufs=CMISC_BUFS) as c_misc,
                    tc.tile_pool(name="c_ps_s", bufs=S_BUFS, space="PSUM") as c_ps_s,
                    tc.tile_pool(name="c_ps_c", bufs=CTX_BUFS, space="PSUM") as c_ps_c,
                ):
                    pend_fin = []
                    for j in range(HPC):
                        pj = slice(64 * j, 64 * (j + 1))
                        for qc in range(QC):
                            def _qproj(qc=qc):
                                qs = slice(qc * 512, (qc + 1) * 512)
                                ps_q = c_ps_s.tile([128, EXP_GROUP, 512], F32,
                                                   tag="s", name=f"psq{qc}")
                                for c in range(8):
                                    nc.tensor.matmul(
                                        ps_q[:, 0, :], wq_sb[:, c, :],
                                        qin[:, c, qs],
                                        start=(c == 0), stop=(c == 7))
                                nc.vector.tensor_scalar_add(
                                    q_pair[:, qc + 1, :], ps_q[:, 0, :],
                                    bq_sb[:])
                            qp = _qproj if (j == 0 and qc + 1 < QC) else None
                            if QPROJ_AT == 0 and qp:
                                qp()
                                qp = None
                            ps_ctx = c_ps_c.tile([128, 512], F32, tag="ctx")
                            rhs_q = q_pair[pj, qc, :]
                            c0 = 0
                            gi = 0
                            while c0 < KC:
                                if gi == FIN_AT and pend_fin:
                                    pend_fin.pop(0)()
                                if gi == QPROJ_AT and qp:
                                    qp()
                                    qp = None
                                gn = min(EXP_GROUP, KC - c0)
                                ps_s = c_ps_s.tile([128, EXP_GROUP, 512], F32,
                                                   tag="s")
                                for cc in range(gn):
                                    lc = c0 + cc
                                    nc.tensor.matmul(
                                        ps_s[:, cc, :],
                                        kT_all[pj,
                                               lc * 128:(lc + 1) * 128],
                                        rhs_q, start=True, stop=True,
                                        tile_position=(64 * j, 0))
                                exp_sb = c_exp.tile([128, EXP_GROUP, 512],
                                                    BF16, tag="e")
                                clean = c0 + gn <= kc_real
                                if clean and EXP_PAT(gi, qc, j):
                                    # exp2 bit-trick: bf16 == i16(x*A16+B16)
                                    nc.vector.tensor_scalar(
                                        exp_sb[:, 0:gn, :].bitcast(I16),
                                        ps_s[:, 0:gn, :], A16, B16,
                                        mybir.AluOpType.mult,
                                        mybir.AluOpType.add)
                                elif clean:
                                    nc.scalar.activation(
                                        exp_sb[:, 0:gn, :], ps_s[:, 0:gn, :],
                                        mybir.ActivationFunctionType.Exp,
                                        bias=0.0, scale=0.125)
                                else:
                                    for cc in range(gn):
                                        nc.scalar.activation(
                                            exp_sb[:, cc, :], ps_s[:, cc, :],
                                            mybir.ActivationFunctionType.Exp,
                                            bias=pb_sb[:, c0 + cc:c0 + cc + 1],
                                            scale=0.125)
                                for cc in range(gn):
                                    lc = c0 + cc
                                    nc.tensor.matmul(
                                        ps_ctx[0:DH + 1, :],
                                        v_all[:, lc, j, :],
                                        exp_sb[:, cc, :],
                                        start=(lc == 0),
                                        stop=(lc == KC - 1))
                                c0 += gn
                                gi += 1

                            def _fin(j=j, qc=qc, ps_ctx=ps_ctx):
                                recip = c_misc.tile([1, 512], BF16,
                                                    tag="recip", name="rc")
                                with nc.allow_low_precision(reason="1/d"):
                                    nc.vector.reciprocal(
                                        recip[:], ps_ctx[DH:DH + 1, :])
                                nc.tensor.matmul(
                                    ps_ctx[64:128, :],
                                    ones_bf[0:1, 0:64], recip[:],
                                    start=True, stop=True,
                                    tile_position=(0, 64),
                                    skip_group_check=True)
                                rec_bc = c_misc.tile([64, 512], F32,
                                                     tag="rbc", name="rb")
                                nc.scalar.copy(rec_bc[:], ps_ctx[64:128, :])
                                ctx_sb = c_misc.tile([64, 512], BF16,
                                                     tag="ctxs", name="cx")
                                nc.vector.tensor_mul(ctx_sb[:],
                                                     ps_ctx[0:64, :],
                                                     rec_bc[:])
                                nc.sync.dma_start(a2a_in[j][qc], ctx_sb[:])
                            if DEFER_FIN:
                                pend_fin.append(_fin)
                            else:
                                _fin()

                        while pend_fin:
                            pend_fin.pop(0)()
                        if "cclocal" in ablate:
                            nc.sync.dma_start(a2a_out[j][:], a2a_in[j][:])
                        else:
                            nc.gpsimd.collective_compute(
                                "AllToAll", mybir.AluOpType.bypass,
                                replica_groups=[list(range(N_CORES))],
                                ins=[a2a_in[j].opt()],
                                outs=[a2a_out[j].opt()])

                # ---------- Phase D: output projection (own 512 rows) ----------
                if "noD" in ablate:
                    continue
                with (
                    tc.tile_pool(name="d_w", bufs=1) as d_w,
                    tc.tile_pool(name="d_out", bufs=3) as d_out,
                    tc.tile_pool(name="d_ps", bufs=8, space="PSUM") as d_ps,
                ):
                    # heads of equal j stacked in pairs on partitions (K=128);
                    # D0 (j=0 pairs) depends only on A2A_0 and overlaps A2A_1.
                    # The 8 (qc, eh) PSUM tiles stay open across both j passes.
                    ctx_p = [d_w.tile([128, 4, 512], BF16, name=f"cxp{j}")
                             for j in range(HPC)]
                    zz = d_w.tile([128, 128], BF16)
                    nc.vector.memset(zz[:], 0.0)
                    ps_os = {}
                    def warm_keeper():
                        for i in range(N_WARM):
                            nc.tensor.matmul(
                                ps_os[(0, 0)][:], zz[:],
                                qin[:, i % 8, 0:512],
                                start=False, stop=False)
                    for j in range(HPC):
                        if j == 1:
                            warm_keeper()
                        ev = a2a_out[j].rearrange("(a two) p q -> a two p q",
                                                  two=2)
                        if j == 0 or not CTXP_SPLIT:
                            nc.sync.dma_start(
                                ctx_p[j][0:64, :, :],
                                ev[:, 0].rearrange("a p q -> p a q"))
                            nc.sync.dma_start(
                                ctx_p[j][64:128, :, :],
                                ev[:, 1].rearrange("a p q -> p a q"))
                        else:
                            for qq in range(4):
                                qs = slice(qq * 128, (qq + 1) * 128)
                                nc.sync.dma_start(
                                    ctx_p[j][0:64, :, qs],
                                    ev[:, 0, :, qs].rearrange(
                                        "a p q -> p a q"))
                                nc.scalar.dma_start(
                                    ctx_p[j][64:128, :, qs],
                                    ev[:, 1, :, qs].rearrange(
                                        "a p q -> p a q"))
                        for qc in range(SQ // 128):
                            for eh in range(2):
                                es = slice(eh * 512, (eh + 1) * 512)
                                if j == 0:
                                    ps_o = d_ps.tile([128, 512], F32,
                                                     tag="pso",
                                                     name=f"po{qc}{eh}")
                                    ps_os[(qc, eh)] = ps_o
                                else:
                                    ps_o = ps_os[(qc, eh)]
                                for a in range(4):
                                    nc.tensor.matmul(
                                        ps_o[:],
                                        ctx_p[j][:, a,
                                                 qc * 128:(qc + 1) * 128],
                                        wo_sb2[:, 4 * j + a, es],
                                        start=(j == 0 and a == 0),
                                        stop=(j == 1 and a == 3))
                                if j == 0:
                                    nc.tensor.matmul(
                                        ps_o[:], ones_bf[:, 0:128],
                                        bo_sb[:, es], start=False, stop=False)
                                else:
                                    o_sb = d_out.tile([128, 512], F32,
                                                      tag="osb")
                                    if (qc + eh) % 2:
                                        nc.vector.tensor_copy(o_sb[:],
                                                              ps_o[:])
                                    else:
                                        nc.scalar.copy(o_sb[:], ps_o[:])
                                    nc.sync.dma_start(
                                        out[qc * 128:(qc + 1) * 128, es],
                                        o_sb[:])

    nc.compile()
    return nc


def prepare(query, key, value, mask, Wq, bq, Wk, bk, Wv, bv, Wo, bo):
    """Host-side sharding/preprocessing + program build. Returns the compiled
    Bass program and the per-core input maps."""
    query = np.asarray(query)
    key = np.asarray(key)
    value = np.asarray(value)
    mask = np.asarray(mask)
    Wq, bq = np.asarray(Wq), np.asarray(bq)
    Wk, bk = np.asarray(Wk), np.asarray(bk)
    Wv, bv = np.asarray(Wv), np.asarray(bv)
    Wo, bo = np.asarray(Wo), np.asarray(bo)

    idx = np.nonzero(mask.reshape(-1) != 0)[0]
    n = int(idx.size)
    n_pad = max(512, ((n + 127) // 128) * 128)
    KC = n_pad // 128
    kc_real = n // 128
    kc_mixed = 1 if n % 128 else 0

    key_c = np.zeros((n_pad, D), np.float32)
    val_c = np.zeros((n_pad, D), np.float32)
    key_c[:n] = key[0, idx, :]
    val_c[:n] = value[0, idx, :]

    kcT_np = _bf16(key_c.T)
    vcT_np = _bf16(val_c.T)
    qT_np = _bf16(query[0].T)
    wqT_np = _bf16(Wq.T)
    wkT_np = _bf16(Wk.T)
    wvT_np = _bf16(Wv.T)
    woT_r = Wo.T
    slots = []
    for j in range(HPC):
        for a in range(4):
            hA, hB = 4 * a + j, 4 * a + 2 + j
            slots.append(woT_r[64 * hA:64 * hA + 64, :])
            slots.append(woT_r[64 * hB:64 * hB + 64, :])
    woT_np = _bf16(np.concatenate(slots, axis=0))
    bo_r_np = _bf16(bo.reshape(1, D))

    pb_np = np.zeros((128, KC), np.float32)
    flat = np.full(n_pad, NEG_BIG, np.float32)
    flat[:n] = 0.0
    pb_np[:] = flat.reshape(KC, 128).T

    nc = build_program(n_pad, kc_real, kc_mixed)

    in_maps = []
    for m in range(N_CORES):
        sl = slice(m * 128, (m + 1) * 128)
        in_maps.append({
            "qT": qT_np,
            "kcT": kcT_np,
            "vcT": vcT_np,
            "wqT": np.ascontiguousarray(wqT_np[:, sl]),
            "wkT": np.ascontiguousarray(wkT_np[:, sl]),
            "wvT": np.ascontiguousarray(wvT_np[:, sl]),
            "woT": woT_np,
            "bq_m": np.ascontiguousarray(
                bq[sl].reshape(128, 1).astype(np.float32)),
            "bk_m": np.ascontiguousarray(
                bk[sl].reshape(128, 1).astype(np.float32)),
            "bv_m": _bf16(bv[sl].reshape(1, 128)),
            "bo_r": bo_r_np,
            "pbias": pb_np,
        })

    return {"nc": nc, "in_maps": in_maps, "n": n, "n_pad": n_pad}


def kernel(query, key, value, mask, Wq, bq, Wk, bk, Wv, bv, Wo, bo,
           _trace=False, _result_box=None):
    prep = prepare(query, key, value, mask, Wq, bq, Wk, bk, Wv, bv, Wo, bo)
    res = run_bass_kernel_spmd(prep["nc"], prep["in_maps"],
                               list(range(N_CORES)), trace=_trace)
    if _result_box is not None:
        _result_box.append(res)

    out = np.concatenate([res.results[m]["out"] for m in range(N_CORES)],
                         axis=0)
    return out.reshape(1, S, D).astype(np.float32)

